# revision 1
# baseline (speedup 1.0000x reference)
"""Trainium2 Bass kernel for DAResBlock3D (dual-attention residual block).

Strategy (8 NeuronCores, SPMD):
  - Spatial sharding over H: core i owns output h-slabs {2i, 2i+1} (512 of
    4096 positions per batch), both batches on-chip as partition halves.
  - 3x3x3 convs: 27 shifted matmuls over a zero-padded local view (4 h-slabs
    with halo), fp32r (full-rate) with 4-way PE packing: row groups = batch,
    col groups = even/odd kernel offset.
  - BatchNorm (train-mode, global stats): per-core partial sums AllGathered
    (1KB) and reduced redundantly on every core.
  - PAM: energy computed transposed (E^T tiles, m on partitions); softmax
    without max-subtraction (energies are small); exp on ScalarE in
    (128,1024) chunks; O = v @ A^T via augmented v^T (ones column gives the
    softmax denominator for free).
  - CAM: per-core partial Gram (64x64) AllGathered; softmax redundant.
  - Cross-core data: AllGather collectives through DRAM bounce buffers; the
    core-dependent halo reads use register-offset DMAs (offset values are
    host-provided per-core inputs).
"""

import os
import sys

sys.path.insert(0, "/opt/trn_rl_repo")

import numpy as np

import concourse.bass as bass
import concourse.mybir as mybir
import concourse.tile as tile
from concourse import bacc
from concourse.bass_utils import run_bass_kernel_spmd
from concourse.masks import make_identity

F32 = mybir.dt.float32
F32R = mybir.dt.float32r
BF16 = mybir.dt.bfloat16
U32 = mybir.dt.uint32
AF = mybir.ActivationFunctionType
ALU = mybir.AluOpType
AX = mybir.AxisListType

NCORES = 8
B = 2
C = 64
HH = 16
N = HH * HH * HH  # 4096
ROW = 18 * 18  # 324, one padded h-slab (w,d padded to 18x18)
MARG = 343  # max |delta| for a 3x3x3 offset = 324+18+1
LOCPAD = 19  # only w/d deltas (+-18, +-1) can underflow a slab base
LOCVIEW = LOCPAD + 4 * ROW + LOCPAD  # local act view: 4 h-slabs + margins
SLAB = 256  # interior positions per h-slab (16x16)
SHARD = 2 * SLAB  # 512 interior positions per batch per core
SLOPE = (1.0 / 8.0 + 1.0 / 3.0) / 2.0  # RReLU eval negative slope
EPS = 1e-5
NTOT = B * N  # BN normalization count = 8192

# DRAM guarded-gather geometry
ACT_ROWSTRIDE = B * C * SLAB  # 32768 elems per h-slab row (64-ch acts)
FCAT_ROWSTRIDE = B * 2 * C * SLAB  # 65536 elems per h-slab row (128-ch)
AG2_S1 = 2 * B * C * SLAB  # 65536: s1 region elems per rank
AG2_GRAM = B * C * C  # 8192: gram region elems per rank
AG2_PER = AG2_S1 + AG2_GRAM  # 73728


def _deltas():
    out = []
    for dh in (-1, 0, 1):
        for dw in (-1, 0, 1):
            for dd in (-1, 0, 1):
                out.append(dh * ROW + dw * 18 + dd)
    return out


DELTAS = _deltas()


def interior_ap(nslab):
    """Free-dim AP picking the 16x16 interior of nslab padded h-slabs."""
    return [[ROW, nslab], [18, 16], [1, 16]]


def build_program():
    nc = bacc.Bacc(
        "TRN2",
        target_bir_lowering=False,
        debug=False,
        num_devices=NCORES,
    )

    # ---- external inputs (per-core in_maps) ----
    x_loc = nc.dram_tensor("x_loc", [128, LOCVIEW], BF16, kind="ExternalInput")
    w_s = nc.dram_tensor("w_s", [128, 27, 64], BF16, kind="ExternalInput")
    w_c = nc.dram_tensor("w_c", [128, 27, 64], BF16, kind="ExternalInput")
    w_s1 = nc.dram_tensor("w_s1", [128, 27, 64], BF16, kind="ExternalInput")
    w_c1 = nc.dram_tensor("w_c1", [128, 27, 64], BF16, kind="ExternalInput")
    w_f = nc.dram_tensor("w_f", [128, 27, 64], BF16, kind="ExternalInput")
    qw_d = nc.dram_tensor("qw", [65, 64], BF16, kind="ExternalInput")
    kw_d = nc.dram_tensor("kw", [65, 64], BF16, kind="ExternalInput")
    vw_d = nc.dram_tensor("vw", [65, 66], BF16, kind="ExternalInput")
    bnp_d = nc.dram_tensor("bnp", [64, 10], F32, kind="ExternalInput")
    gam_d = nc.dram_tensor("gam", [1, 2], F32, kind="ExternalInput")
    out_d = nc.dram_tensor("out", [B, C, SHARD], F32, kind="ExternalOutput")

    rg = [list(range(NCORES))]

    with tile.TileContext(nc) as tc:
        dram_cm = tc.tile_pool(name="dram", bufs=1, space="DRAM")
        dram = dram_cm.__enter__()
        # collective bounce buffers
        st1_in = dram.tile([64, 4], F32)
        st1_out = dram.tile([NCORES, 64, 4], F32, addr_space="Shared")
        ag2_in = dram.tile([AG2_PER], F32)
        ag2_out = dram.tile([NCORES * AG2_PER], F32, addr_space="Shared")
        c2_in = dram.tile([2, B, C, SLAB], BF16)
        c2_ag = dram.tile([NCORES, 2, B, C, SLAB], BF16, addr_space="Shared")
        c2_ri = dram.tile([NCORES, 2, B, C, SLAB], BF16)
        c2_ro = dram.tile([2, B, C, SLAB], BF16)
        s2_in = dram.tile([2, B, C, SLAB], BF16)
        s2_ag = dram.tile([NCORES, 2, B, C, SLAB], BF16, addr_space="Shared")
        s2_ri = dram.tile([NCORES, 2, B, C, SLAB], BF16)
        s2_ro = dram.tile([2, B, C, SLAB], BF16)
        st2_in = dram.tile([64, 4], F32)
        st2_out = dram.tile([NCORES, 64, 4], F32, addr_space="Shared")
        fc_in = dram.tile([2, B, 2 * C, SLAB], BF16)
        fc_ag = dram.tile([NCORES, 2, B, 2 * C, SLAB], BF16, addr_space="Shared")
        fc_ri = dram.tile([NCORES, 2, B, 2 * C, SLAB], BF16)
        fc_ro = dram.tile([2, B, 2 * C, SLAB], BF16)
        stf_in = dram.tile([64, 2], F32)
        stf_out = dram.tile([NCORES, 64, 2], F32, addr_space="Shared")
        bcast_dram = dram.tile([B, SHARD], F32)

        singles_cm = tc.tile_pool(name="singles", bufs=1)
        singles = singles_cm.__enter__()

        ident = singles.tile([64, 64], BF16)
        make_identity(nc, ident[:])
        ident_f32 = singles.tile([64, 64], F32)
        make_identity(nc, ident_f32[:])

        # constants to SBUF
        qw_sb = singles.tile([65, 64], BF16)
        kw_sb = singles.tile([65, 64], BF16)
        vw_sb = singles.tile([65, 66], BF16)
        bnp = singles.tile([64, 10], F32)
        gam_p = singles.tile([1, 1], F32)
        gam_c_col = singles.tile([64, 1], F32)
        ones_row = singles.tile([1, 64], F32)
        nc.sync.dma_start(out=qw_sb[:], in_=qw_d[:])
        nc.sync.dma_start(out=kw_sb[:], in_=kw_d[:])
        nc.sync.dma_start(out=vw_sb[:], in_=vw_d[:])
        nc.sync.dma_start(out=bnp[:], in_=bnp_d[:])
        nc.sync.dma_start(out=gam_p[:], in_=gam_d[0:1, 0:1])
        nc.sync.dma_start(
            out=gam_c_col[:],
            in_=bass.AP(tensor=gam_d, offset=1, ap=[[0, 64], [1, 1]]),
        )
        nc.vector.memset(ones_row[:], 1.0)
        eps_col = singles.tile([64, 1], F32)
        nc.vector.memset(eps_col[:], EPS)
        zrow = singles.tile([128, SLAB], BF16)
        nc.vector.memset(zrow[:], 0.0)


        # big persistent activations
        acts_cm = tc.tile_pool(name="acts", bufs=1)
        acts = acts_cm.__enter__()
        x_sb = acts.tile([128, LOCVIEW], BF16)
        nc.sync.dma_start(out=x_sb[:], in_=x_loc[:])

        s1_own = [acts.tile([65, SHARD], F32, name=f"s1own{b}") for b in range(B)]
        s1_own_bf = [acts.tile([65, SHARD], BF16, name=f"s1ownbf{b}") for b in range(B)]
        c1_own = [acts.tile([64, SHARD], F32, name=f"c1own{b}") for b in range(B)]
        c1_own_bf = [acts.tile([64, SHARD], BF16, name=f"c1ownbf{b}") for b in range(B)]
        for b in range(B):
            nc.vector.memset(s1_own[b][64:65, :], 1.0)
            nc.vector.memset(s1_own_bf[b][64:65, :], 1.0)

        s1_pam = [acts.tile([65, N], BF16, name=f"s1pam{b}") for b in range(B)]
        for b in range(B):
            nc.vector.memset(s1_pam[b][64:65, :], 1.0)

        k_stack = acts.tile([128, N], BF16)
        q_stack = acts.tile([128, SHARD], BF16)
        vt_sb = [acts.tile([128, 32 * 66], BF16, name=f"vt{b}") for b in range(B)]

        wpool_cm = tc.tile_pool(name="wpool", bufs=2)
        wpool = wpool_cm.__enter__()

        stats_pool_cm = tc.tile_pool(name="stats", bufs=1)
        stats_pool = stats_pool_cm.__enter__()

        tmp_pool_cm = tc.tile_pool(name="tmp", bufs=2)
        tmp_pool = tmp_pool_cm.__enter__()

        # ---------------- helpers ----------------
        def load_wconv(dram_t, name):
            w = wpool.tile([128, 27, 64], BF16, tag="wconv", name=name)
            nc.sync.dma_start(out=w[:], in_=dram_t[:])
            return w

        def conv64(w_sb_t, act, psum_pool, tname):
            """3x3x3 conv over 64-ch padded local view for own 2 slabs.

            Returns per-batch compact raw-output tiles t[b] (64, 512) plus
            (sum, sumsq) stat columns (64,1) each."""
            touts = []
            stats = []
            for b in range(B):
                t = stats_pool.tile([64, SHARD], F32, name=f"{tname}_t{b}")
                for jj, jslab in enumerate((1, 2)):
                    ps = psum_pool.tile(
                        [64, ROW], F32, tag=f"convps{b}", name=f"{tname}ps{b}{jj}"
                    )
                    base = LOCPAD + jslab * ROW
                    for o in range(27):
                        nc.tensor.matmul(
                            ps[:],
                            lhsT=w_sb_t[64 * b : 64 * b + 64, o, :],
                            rhs=act[
                                64 * b : 64 * b + 64,
                                base + DELTAS[o] : base + DELTAS[o] + ROW,
                            ],
                            start=(o == 0),
                            stop=(o == 26),
                            tile_position=(64 * b, 0),
                        )
                    nc.vector.tensor_copy(
                        t[:, jj * SLAB : (jj + 1) * SLAB],
                        ps[:, :].rearrange("p (w d) -> p w d", w=18)[
                            :, 1:17, 1:17
                        ],
                    )
                touts.append(t)
                ssum = stats_pool.tile([64, 1], F32, name=f"{tname}_s{b}")
                ssq = stats_pool.tile([64, 1], F32, name=f"{tname}_q{b}")
                scr2 = tmp_pool.tile([64, SHARD], F32, tag="scrB", name=f"{tname}scrB{b}")
                nc.vector.reduce_sum(ssum[:], t[:], axis=AX.X)
                nc.scalar.activation(scr2[:], t[:], AF.Square, accum_out=ssq[:])
                stats.append((ssum, ssq))
            return touts, stats

        def conv128(w_sb_t, act_pair, psum_pool, tname):
            """3x3x3 conv with 128 input channels (fused concat), per batch."""
            touts = []
            stats = []
            for b in range(B):
                t = stats_pool.tile([64, SHARD], F32, name=f"{tname}_t{b}")
                for jj, jslab in enumerate((1, 2)):
                    ps = psum_pool.tile(
                        [64, ROW], F32, tag=f"convps{b}", name=f"{tname}ps{b}{jj}"
                    )
                    base = LOCPAD + jslab * ROW
                    for o in range(27):
                        nc.tensor.matmul(
                            ps[:],
                            lhsT=w_sb_t[:, o, :],
                            rhs=act_pair[b][
                                :, base + DELTAS[o] : base + DELTAS[o] + ROW
                            ],
                            start=(o == 0),
                            stop=(o == 26),
                        )
                    nc.vector.tensor_copy(
                        t[:, jj * SLAB : (jj + 1) * SLAB],
                        ps[:, :].rearrange("p (w d) -> p w d", w=18)[
                            :, 1:17, 1:17
                        ],
                    )
                touts.append(t)
                ssum = stats_pool.tile([64, 1], F32, name=f"{tname}_s{b}")
                ssq = stats_pool.tile([64, 1], F32, name=f"{tname}_q{b}")
                scr2 = tmp_pool.tile([64, SHARD], F32, tag="scrB", name=f"{tname}scrB{b}")
                nc.vector.reduce_sum(ssum[:], t[:], axis=AX.X)
                nc.scalar.activation(scr2[:], t[:], AF.Square, accum_out=ssq[:])
                stats.append((ssum, ssq))
            return touts, stats

        def pack_stats(dst_sb, stats_list):
            """stats_list: list of (ssum_b0, ssq_b0), (ssum_b1, ssq_b1) pairs
            per conv; writes [sum, sq] per conv into dst columns."""
            for ci, st in enumerate(stats_list):
                (s0, q0), (s1_, q1) = st
                nc.vector.tensor_add(dst_sb[:, 2 * ci : 2 * ci + 1], s0[:], s1_[:])
                nc.vector.tensor_add(
                    dst_sb[:, 2 * ci + 1 : 2 * ci + 2], q0[:], q1[:]
                )

        def bn_coeffs(tot_sb, col, g_col, b_col, name):
            """From total [sum, sumsq] cols compute A=(g*rstd), B=b-mean*A and
            the rrelu-scaled variants. Returns (A, B, As, Bs) (64,1) tiles."""
            mean = stats_pool.tile([64, 1], F32, name=f"{name}_mean")
            var = stats_pool.tile([64, 1], F32, name=f"{name}_var")
            a_t = stats_pool.tile([64, 1], F32, name=f"{name}_A")
            b_t = stats_pool.tile([64, 1], F32, name=f"{name}_B")
            as_t = stats_pool.tile([64, 1], F32, name=f"{name}_As")
            bs_t = stats_pool.tile([64, 1], F32, name=f"{name}_Bs")
            scr = stats_pool.tile([64, 1], F32, name=f"{name}_scr")
            nc.vector.tensor_scalar(
                mean[:], tot_sb[:, col : col + 1], 1.0 / NTOT, None, ALU.mult
            )
            nc.vector.tensor_scalar(
                var[:], tot_sb[:, col + 1 : col + 2], 1.0 / NTOT, None, ALU.mult
            )
            nc.vector.tensor_mul(scr[:], mean[:], mean[:])
            nc.vector.tensor_sub(var[:], var[:], scr[:])
            # rstd = exp(-0.5*ln(var+eps)); avoids the Sqrt table set
            nc.scalar.activation(scr[:], var[:], AF.Ln, bias=eps_col[:])
            nc.vector.tensor_scalar(scr[:], scr[:], -0.5, None, ALU.mult)
            nc.scalar.activation(scr[:], scr[:], AF.Exp)
            nc.vector.tensor_mul(a_t[:], scr[:], g_col)
            nc.vector.tensor_mul(scr[:], mean[:], a_t[:])
            nc.vector.tensor_sub(b_t[:], b_col, scr[:])
            nc.vector.tensor_scalar(as_t[:], a_t[:], SLOPE, None, ALU.mult)
            nc.vector.tensor_scalar(bs_t[:], b_t[:], SLOPE, None, ALU.mult)
            return a_t, b_t, as_t, bs_t

        def bn_rrelu(t_raw, coeffs, dst_ap):
            """dst = max(A*t+B, As*t+Bs) elementwise."""
            a_t, b_t, as_t, bs_t = coeffs
            y1 = tmp_pool.tile([64, SHARD], F32, tag="y1", name="y1_t")
            y2 = tmp_pool.tile([64, SHARD], F32, tag="y2", name="y2_t")
            nc.vector.tensor_scalar(
                y1[:], t_raw[:], a_t[:], b_t[:], ALU.mult, ALU.add
            )
            nc.vector.tensor_scalar(
                y2[:], t_raw[:], as_t[:], bs_t[:], ALU.mult, ALU.add
            )
            nc.vector.tensor_max(dst_ap, y1[:], y2[:])

        def halo_exchange(in_t, ag_t, ri_t, ro_t, nch):
            """AG own slabs, then RS-rotate so each core receives exactly its
            lo/hi halo slabs (slot-static reads of the gathered buffer)."""
            nc.gpsimd.collective_compute(
                "AllGather", ALU.bypass, replica_groups=rg,
                ins=[in_t[:].opt()], outs=[ag_t[:].opt()],
            )
            blk = B * nch * SLAB  # one slab block (elements)
            per = 2 * blk  # one rank contribution
            for i in range(NCORES):
                # lo slot: rank i-1's slab 1
                if i > 0:
                    nc.sync.dma_start(
                        out=bass.AP(
                            tensor=ri_t[:].tensor,
                            offset=i * per,
                            ap=[[1, blk]],
                        ),
                        in_=bass.AP(
                            tensor=ag_t[:].tensor,
                            offset=(i - 1) * per + blk,
                            ap=[[1, blk]],
                        ),
                    )
                else:
                    for z in range(blk // (128 * SLAB)):
                        nc.sync.dma_start(
                            out=bass.AP(
                                tensor=ri_t[:].tensor,
                                offset=z * 128 * SLAB,
                                ap=[[SLAB, 128], [1, SLAB]],
                            ),
                            in_=zrow[:],
                        )
                # hi slot: rank i+1's slab 0
                if i < NCORES - 1:
                    nc.sync.dma_start(
                        out=bass.AP(
                            tensor=ri_t[:].tensor,
                            offset=i * per + blk,
                            ap=[[1, blk]],
                        ),
                        in_=bass.AP(
                            tensor=ag_t[:].tensor,
                            offset=(i + 1) * per,
                            ap=[[1, blk]],
                        ),
                    )
                else:
                    for z in range(blk // (128 * SLAB)):
                        nc.sync.dma_start(
                            out=bass.AP(
                                tensor=ri_t[:].tensor,
                                offset=i * per + blk + z * 128 * SLAB,
                                ap=[[SLAB, 128], [1, SLAB]],
                            ),
                            in_=zrow[:],
                        )
            nc.gpsimd.collective_compute(
                "ReduceScatter", ALU.add, replica_groups=rg,
                ins=[ri_t[:].opt()], outs=[ro_t[:].opt()],
            )

        def build_view(ro_t, nch, bsel, dst, own_ap, name):
            """dst (128, LOCVIEW) bf16: slabs 1-2 <- own; 0/3 <- RS halos/8."""
            blk = B * nch * SLAB
            boff = 0 if bsel is None else bsel * nch * SLAB
            for dslab, hs in ((0, 0), (3, 1)):
                stg = tmp_pool.tile(
                    [128, SLAB], BF16, tag="hstg", name=f"hs{name}{dslab}"
                )
                nc.sync.dma_start(
                    out=stg[:],
                    in_=bass.AP(
                        tensor=ro_t[:].tensor,
                        offset=hs * blk + boff,
                        ap=[[SLAB, 128], [1, SLAB]],
                    ),
                )
                nc.vector.tensor_scalar(
                    dst[:, LOCPAD + dslab * ROW : LOCPAD + (dslab + 1) * ROW]
                    .rearrange("p (w d) -> p w d", w=18)[:, 1:17, 1:17],
                    stg[:].rearrange("p (w d) -> p w d", w=16),
                    1.0 / NCORES,
                    None,
                    ALU.mult,
                )
            nc.vector.tensor_copy(
                dst[:, LOCPAD + 1 * ROW : LOCPAD + 3 * ROW]
                .rearrange("p (j w d) -> p j w d", j=2, w=18)[:, :, 1:17, 1:17],
                own_ap,
            )

        # =========== phase 1: conv S and conv C (input x) ===========
        cpsum_cm = tc.tile_pool(name="cpsum", bufs=2, space="PSUM")
        cpsum = cpsum_cm.__enter__()

        ws_sb = load_wconv(w_s, "wsS")
        tS, statS = conv64(ws_sb, x_sb, cpsum, "cS")
        wc_sb = load_wconv(w_c, "wsC")
        tC, statC = conv64(wc_sb, x_sb, cpsum, "cC")

        st1_sb = stats_pool.tile([64, 4], F32)
        pack_stats(st1_sb, [statS, statC])
        nc.sync.dma_start(out=st1_in[:], in_=st1_sb[:])
        nc.gpsimd.collective_compute(
            "AllGather",
            ALU.bypass,
            replica_groups=rg,
            ins=[st1_in[:].opt()],
            outs=[st1_out[:].opt()],
        )

        # reduce gathered stats and compute BN coefficients
        st1_stage = stats_pool.tile([64, 4, NCORES], F32)
        nc.sync.dma_start(
            out=st1_stage[:],
            in_=bass.AP(
                tensor=st1_out[:].tensor,
                offset=0,
                ap=[[4, 64], [1, 4], [256, NCORES]],
            ),
        )
        st1_tot = stats_pool.tile([64, 4], F32)
        nc.vector.tensor_reduce(st1_tot[:], st1_stage[:], axis=AX.X, op=ALU.add)
        cS = bn_coeffs(st1_tot, 0, bnp[:, 0:1], bnp[:, 1:2], "bnS")
        cC = bn_coeffs(st1_tot, 2, bnp[:, 2:3], bnp[:, 3:4], "bnC")

        for b in range(B):
            bn_rrelu(tS[b], cS, s1_own[b][0:64, :])
            bn_rrelu(tC[b], cC, c1_own[b][:, :])
            nc.vector.tensor_copy(s1_own_bf[b][0:64, :], s1_own[b][0:64, :])
            nc.vector.tensor_copy(c1_own_bf[b][:, :], c1_own[b][:, :])

        cpsum_cm.__exit__(None, None, None)

        # =========== phase 2: CAM partial gram + AG2 (s1 + gram) ===========
        mpsum_cm = tc.tile_pool(name="mpsum", bufs=2, space="PSUM")
        mpsum = mpsum_cm.__enter__()

        ft_sb = [tmp_pool.tile([128, 4 * 64], BF16, tag=f"ft{b}", name=f"ft{b}") for b in range(B)]
        gram_sb = tmp_pool.tile([64, B * 64], F32, tag="gram")
        for b in range(B):
            for kk in range(4):
                pst = mpsum.tile([128, 64], BF16, tag="mm", name=f"ft{b}{kk}")
                nc.tensor.transpose(
                    pst[:],
                    c1_own_bf[b][:, 128 * kk : 128 * (kk + 1)],
                    ident[:],
                )
                nc.vector.tensor_copy(
                    ft_sb[b][:, 64 * kk : 64 * (kk + 1)], pst[:, 0:64]
                )
            psg = mpsum.tile([64, 64], F32, tag="mm", name=f"gram{b}")
            for kk in range(4):
                nc.tensor.matmul(
                    psg[:],
                    lhsT=ft_sb[b][:, 64 * kk : 64 * (kk + 1)],
                    rhs=ft_sb[b][:, 64 * kk : 64 * (kk + 1)],
                    start=(kk == 0),
                    stop=(kk == 3),
                )
            nc.vector.tensor_copy(gram_sb[:, 64 * b : 64 * (b + 1)], psg[:])

        # write AG2 contribution: s1 (slab-major) + gram
        for b in range(B):
            nc.sync.dma_start(
                out=bass.AP(
                    tensor=ag2_in[:].tensor,
                    offset=b * C * SLAB,
                    ap=[[SLAB, 64], [B * C * SLAB, 2], [1, SLAB]],
                ),
                in_=s1_own[b][0:64, :].rearrange("p (j s) -> p j s", j=2),
            )
        nc.sync.dma_start(
            out=bass.AP(
                tensor=ag2_in[:].tensor,
                offset=AG2_S1,
                ap=[[64, 64], [64 * 64, B], [1, 64]],
            ),
            in_=gram_sb[:].rearrange("p (b c) -> p b c", b=B),
        )
        nc.gpsimd.collective_compute(
            "AllGather",
            ALU.bypass,
            replica_groups=rg,
            ins=[ag2_in[:].opt()],
            outs=[ag2_out[:].opt()],
        )

        # =========== phase 3: q (local), then k/vT from gathered s1 ===========
        for b in range(B):
            psq = mpsum.tile([64, SHARD], F32, tag="qk", name=f"q{b}")
            nc.tensor.matmul(
                psq[:],
                lhsT=qw_sb[:],
                rhs=s1_own_bf[b][:],
                start=True,
                stop=True,
            )
            nc.vector.tensor_copy(q_stack[64 * b : 64 * (b + 1), :], psq[:])

        # load gathered s1 into s1_pam (global n order); one DMA per slab half
        for b in range(B):
            for j in range(2):
                nc.gpsimd.dma_start(
                    out=s1_pam[b][0:64, :]
                    .rearrange("p (g s) -> p g s", s=2 * SLAB)[:, :, j * SLAB : (j + 1) * SLAB],
                    in_=bass.AP(
                        tensor=ag2_out[:].tensor,
                        offset=b * C * SLAB + j * B * C * SLAB,
                        ap=[[SLAB, 64], [AG2_PER, NCORES], [1, SLAB]],
                    ),
                )
        # gathered gram -> reduce over cores
        gram_full = [tmp_pool.tile([64, 64], F32, tag=f"gramf{b}", name=f"gramf{b}") for b in range(B)]
        for b in range(B):
            gstage = tmp_pool.tile(
                [64, 64, NCORES], F32, tag="gstage", name=f"gstage{b}"
            )
            nc.sync.dma_start(
                out=gstage[:],
                in_=bass.AP(
                    tensor=ag2_out[:].tensor,
                    offset=AG2_S1 + b * C * C,
                    ap=[[64, 64], [1, 64], [AG2_PER, NCORES]],
                ),
            )
            nc.vector.tensor_reduce(gram_full[b][:], gstage[:], axis=AX.X, op=ALU.add)

        for b in range(B):
            for nt in range(8):
                psk = mpsum.tile([64, 512], F32, tag="qk", name=f"k{b}{nt}")
                nc.tensor.matmul(
                    psk[:],
                    lhsT=kw_sb[:],
                    rhs=s1_pam[b][:, 512 * nt : 512 * (nt + 1)],
                    start=True,
                    stop=True,
                )
                nc.vector.tensor_copy(
                    k_stack[64 * b : 64 * (b + 1), 512 * nt : 512 * (nt + 1)],
                    psk[:],
                )
            for mt in range(32):
                psv = mpsum.tile([128, 66], F32, tag="vt", name=f"v{b}{mt}")
                nc.tensor.matmul(
                    psv[:],
                    lhsT=s1_pam[b][:, 128 * mt : 128 * (mt + 1)],
                    rhs=vw_sb[:],
                    start=True,
                    stop=True,
                )
                nc.vector.tensor_copy(
                    vt_sb[b][:, 66 * mt : 66 * (mt + 1)], psv[:]
                )

        # =========== phase 4: CAM finish -> c2 -> pair halo AG ===========
        c2both = acts.tile([128, SHARD], BF16)
        for b in range(B):
            rowmax = tmp_pool.tile([64, 1], F32, tag="camx", name=f"camx{b}")
            den = tmp_pool.tile([64, 1], F32, tag="camd", name=f"camd{b}")
            attn = tmp_pool.tile([64, 64], F32, tag="cama", name=f"cama{b}")
            nc.vector.tensor_reduce(
                rowmax[:], gram_full[b][:], axis=AX.X, op=ALU.min
            )
            nc.scalar.activation(
                attn[:],
                gram_full[b][:],
                AF.Exp,
                bias=rowmax[:],
                scale=-1.0,
                accum_out=den[:],
            )
            nc.vector.reciprocal(den[:], den[:])
            nc.vector.tensor_scalar(attn[:], attn[:], den[:], None, ALU.mult)
            # attn^T via PE
            psat = mpsum.tile([64, 64], F32, tag="mm", name=f"at{b}")
            nc.tensor.transpose(psat[:], attn[:], ident_f32[:])
            attnT = tmp_pool.tile([64, 64], BF16, tag="camat", name=f"camat{b}")
            nc.vector.tensor_copy(attnT[:], psat[:])
            # cam_out = attnT.T @ c1_own
            psco = mpsum.tile([64, SHARD], F32, tag="qk", name=f"co{b}")
            nc.tensor.matmul(
                psco[:],
                lhsT=attnT[:],
                rhs=c1_own_bf[b][:],
                start=True,
                stop=True,
            )
            c2t = tmp_pool.tile([64, SHARD], F32, tag="c2t", name=f"c2t{b}")
            nc.vector.tensor_scalar(c2t[:], psco[:], gam_c_col[:, 0:1], None, ALU.mult)
            nc.vector.tensor_add(
                c2both[64 * b : 64 * (b + 1), :], c2t[:], c1_own[b][:]
            )
            nc.sync.dma_start(
                out=bass.AP(
                    tensor=c2_in[:].tensor,
                    offset=b * C * SLAB,
                    ap=[[SLAB, 64], [B * C * SLAB, 2], [1, SLAB]],
                ),
                in_=c2both[64 * b : 64 * (b + 1), :].rearrange(
                    "p (j s) -> p j s", j=2
                ),
            )
        halo_exchange(c2_in, c2_ag, c2_ri, c2_ro, C)

        mpsum_cm.__exit__(None, None, None)

        # =========== phase 5: PAM attention ===========
        epsum_cm = tc.tile_pool(name="epsum", bufs=3, space="PSUM")
        epsum = epsum_cm.__enter__()
        opsum_cm = tc.tile_pool(name="opsum", bufs=1, space="PSUM")
        opsum = opsum_cm.__enter__()
        apool_cm = tc.tile_pool(name="apool", bufs=3)
        apool = apool_cm.__enter__()

        o_ps = [
            opsum.tile([65, SHARD], F32, name=f"ops{b}", tag=f"ops{b}")
            for b in range(B)
        ]
        for g2 in range(16):
            for b in range(B):
                e_ps = epsum.tile([128, 1024], F32, tag="eg", name=f"e{g2}{b}")
                for j in range(2):
                    mt = 2 * g2 + j
                    nc.tensor.matmul(
                        e_ps[:, 512 * j : 512 * (j + 1)],
                        lhsT=k_stack[
                            64 * b : 64 * (b + 1), 128 * mt : 128 * (mt + 1)
                        ],
                        rhs=q_stack[64 * b : 64 * (b + 1), :],
                        start=True,
                        stop=True,
                        tile_position=(64 * b, 0),
                    )
                a_sb = apool.tile([128, 1024], BF16, tag="ag", name=f"a{g2}{b}")
                nc.scalar.activation(a_sb[:], e_ps[:], AF.Exp)
                for j in range(2):
                    mt = 2 * g2 + j
                    nc.tensor.matmul(
                        o_ps[b][:],
                        lhsT=vt_sb[b][:, 66 * mt : 66 * mt + 65],
                        rhs=a_sb[:, 512 * j : 512 * (j + 1)],
                        start=(mt == 0),
                        stop=(mt == 31),
                    )

        # =========== phase 6: PAM finalize -> s2 -> pair halo AG ===========
        s2both = acts.tile([128, SHARD], BF16)
        for b in range(B):
            recip = tmp_pool.tile([1, SHARD], F32, tag="rec", name=f"rec{b}")
            recipg = tmp_pool.tile([1, SHARD], F32, tag="recg", name=f"recg{b}")
            nc.vector.reciprocal(recip[:], o_ps[b][64:65, :])
            nc.vector.tensor_scalar(
                recipg[:], recip[:], gam_p[:, 0:1], None, ALU.mult
            )
            nc.sync.dma_start(out=bcast_dram[b : b + 1, :], in_=recipg[:])
            bc_sb = tmp_pool.tile([64, SHARD], F32, tag="bcs", name=f"bcs{b}")
            nc.sync.dma_start(
                out=bc_sb[:],
                in_=bass.AP(
                    tensor=bcast_dram[:].tensor,
                    offset=b * SHARD,
                    ap=[[0, 64], [1, SHARD]],
                ),
            )
            s2t = tmp_pool.tile([64, SHARD], F32, tag="s2t", name=f"s2t{b}")
            nc.vector.tensor_mul(s2t[:], o_ps[b][0:64, :], bc_sb[:])
            nc.vector.tensor_add(
                s2both[64 * b : 64 * (b + 1), :], s2t[:], s1_own[b][0:64, :]
            )
            nc.sync.dma_start(
                out=bass.AP(
                    tensor=s2_in[:].tensor,
                    offset=b * C * SLAB,
                    ap=[[SLAB, 64], [B * C * SLAB, 2], [1, SLAB]],
                ),
                in_=s2both[64 * b : 64 * (b + 1), :].rearrange(
                    "p (j s) -> p j s", j=2
                ),
            )
        halo_exchange(s2_in, s2_ag, s2_ri, s2_ro, C)

        for p in (apool_cm, opsum_cm, epsum_cm):
            p.__exit__(None, None, None)
        cpsum_cm = tc.tile_pool(name="cpsum2", bufs=2, space="PSUM")
        cpsum = cpsum_cm.__enter__()

        # =========== phase 7: conv C1 (on gathered c2) ===========
        c2_loc = acts.tile([128, LOCVIEW], BF16)
        nc.gpsimd.memset(c2_loc[:], 0.0)
        build_view(
            c2_ro, C, None, c2_loc,
            c2both[:].rearrange("p (j w d) -> p j w d", j=2, w=16), "c2",
        )
        wc1_sb = load_wconv(w_c1, "wsC1")
        tC1, statC1 = conv64(wc1_sb, c2_loc, cpsum, "cC1")

        # =========== phase 8: conv S1 (on gathered s2) ===========
        s2_loc = acts.tile([128, LOCVIEW], BF16)
        nc.gpsimd.memset(s2_loc[:], 0.0)
        build_view(
            s2_ro, C, None, s2_loc,
            s2both[:].rearrange("p (j w d) -> p j w d", j=2, w=16), "s2",
        )
        ws1_sb = load_wconv(w_s1, "wsS1")
        tS1, statS1 = conv64(ws1_sb, s2_loc, cpsum, "cS1")

        st2_sb = stats_pool.tile([64, 4], F32)
        pack_stats(st2_sb, [statS1, statC1])
        nc.sync.dma_start(out=st2_in[:], in_=st2_sb[:])
        nc.gpsimd.collective_compute(
            "AllGather",
            ALU.bypass,
            replica_groups=rg,
            ins=[st2_in[:].opt()],
            outs=[st2_out[:].opt()],
        )
        st2_stage = stats_pool.tile([64, 4, NCORES], F32)
        nc.sync.dma_start(
            out=st2_stage[:],
            in_=bass.AP(
                tensor=st2_out[:].tensor,
                offset=0,
                ap=[[4, 64], [1, 4], [256, NCORES]],
            ),
        )
        st2_tot = stats_pool.tile([64, 4], F32)
        nc.vector.tensor_reduce(st2_tot[:], st2_stage[:], axis=AX.X, op=ALU.add)
        cS1 = bn_coeffs(st2_tot, 0, bnp[:, 4:5], bnp[:, 5:6], "bnS1")
        cC1 = bn_coeffs(st2_tot, 2, bnp[:, 6:7], bnp[:, 7:8], "bnC1")

        fcat_own = acts.tile([128, B * SHARD], BF16)
        for b in range(B):
            bn_rrelu(tS1[b], cS1, fcat_own[0:64, b * SHARD : (b + 1) * SHARD])
            bn_rrelu(tC1[b], cC1, fcat_own[64:128, b * SHARD : (b + 1) * SHARD])

        # contribution: (2 slab, 2 b, 128 c, 256); one DMA per batch
        for b in range(B):
            nc.sync.dma_start(
                out=bass.AP(
                    tensor=fc_in[:].tensor,
                    offset=b * 2 * C * SLAB,
                    ap=[[SLAB, 128], [B * 2 * C * SLAB, 2], [1, SLAB]],
                ),
                in_=fcat_own[:, b * SHARD : (b + 1) * SHARD].rearrange(
                    "p (j s) -> p j s", j=2
                ),
            )
        halo_exchange(fc_in, fc_ag, fc_ri, fc_ro, 2 * C)

        # =========== phase 9: conv F ===========
        fcat_loc = [acts.tile([128, LOCVIEW], BF16, name=f"fl{b}") for b in range(B)]
        for b in range(B):
            nc.gpsimd.memset(fcat_loc[b][:], 0.0)
            build_view(
                fc_ro, 2 * C, b, fcat_loc[b],
                fcat_own[:, b * SHARD : (b + 1) * SHARD].rearrange(
                    "p (j w d) -> p j w d", j=2, w=16
                ),
                f"fc{b}",
            )
        wf_sb = load_wconv(w_f, "wsF")
        tF, statF = conv128(wf_sb, fcat_loc, cpsum, "cF")

        stf_sb = stats_pool.tile([64, 2], F32)
        pack_stats(stf_sb, [statF])
        nc.sync.dma_start(out=stf_in[:], in_=stf_sb[:])
        nc.gpsimd.collective_compute(
            "AllGather",
            ALU.bypass,
            replica_groups=rg,
            ins=[stf_in[:].opt()],
            outs=[stf_out[:].opt()],
        )
        stf_stage = stats_pool.tile([64, 2, NCORES], F32)
        nc.sync.dma_start(
            out=stf_stage[:],
            in_=bass.AP(
                tensor=stf_out[:].tensor,
                offset=0,
                ap=[[2, 64], [1, 2], [128, NCORES]],
            ),
        )
        stf_tot = stats_pool.tile([64, 2], F32)
        nc.vector.tensor_reduce(stf_tot[:], stf_stage[:], axis=AX.X, op=ALU.add)
        cF = bn_coeffs(stf_tot, 0, bnp[:, 8:9], bnp[:, 9:10], "bnF")

        out_own = acts.tile([128, SHARD], F32)
        for b in range(B):
            bn_rrelu(tF[b], cF, out_own[64 * b : 64 * (b + 1), :])
        nc.sync.dma_start(
            out=bass.AP(
                tensor=out_d,
                offset=0,
                ap=[[SHARD, 128], [1, SHARD]],
            ),
            in_=out_own[:],
        )

        for p in (cpsum_cm, tmp_pool_cm, stats_pool_cm, wpool_cm, acts_cm,
                  singles_cm, dram_cm):
            p.__exit__(None, None, None)

    nc.finalize()
    return nc




def _prep_host(inputs):
    """Build per-core in_maps from the full problem inputs."""
    x = np.asarray(inputs["x"], np.float32)

    import ml_dtypes

    def conv_wT(w):
        # w: (O, I, 3, 3, 3) -> (128, 27, 64): [dup*64+i, off, o]
        wt = np.transpose(np.asarray(w, np.float32), (1, 2, 3, 4, 0)).reshape(
            w.shape[1], 27, 64
        )
        if w.shape[1] == 64:
            wt = np.concatenate([wt, wt], axis=0)
        return wt.astype(ml_dtypes.bfloat16)

    qw = np.asarray(inputs["qw"], np.float32).reshape(64, 64)
    kw = np.asarray(inputs["kw"], np.float32).reshape(64, 64)
    vw = np.asarray(inputs["vw"], np.float32).reshape(64, 64)
    qa = np.zeros((65, 64), np.float32)
    qa[:64] = qw.T
    qa[64] = np.asarray(inputs["qb"], np.float32)
    ka = np.zeros((65, 64), np.float32)
    ka[:64] = kw.T
    ka[64] = np.asarray(inputs["kb"], np.float32)
    va = np.zeros((65, 66), np.float32)
    va[:64, :64] = vw.T
    va[64, :64] = np.asarray(inputs["vb"], np.float32)
    va[64, 64] = 1.0

    bnp = np.stack(
        [
            np.asarray(inputs[k], np.float32)
            for k in ("gS", "bS", "gC", "bC", "gS1", "bS1", "gC1", "bC1", "gF", "bF")
        ],
        axis=1,
    )
    gam = np.array(
        [[float(np.asarray(inputs["gamma_p"]).reshape(-1)[0]),
          float(np.asarray(inputs["gamma_c"]).reshape(-1)[0])]],
        np.float32,
    )

    shared = {
        "w_s": conv_wT(inputs["wS"]),
        "w_c": conv_wT(inputs["wC"]),
        "w_s1": conv_wT(inputs["wS1"]),
        "w_c1": conv_wT(inputs["wC1"]),
        "w_f": conv_wT(inputs["wF"]),
        "qw": qa.astype(ml_dtypes.bfloat16),
        "kw": ka.astype(ml_dtypes.bfloat16),
        "vw": va.astype(ml_dtypes.bfloat16),
        "bnp": np.ascontiguousarray(bnp),
        "gam": gam,
    }

    # padded full volume (h, w, d) -> (18, 18, 18), flattened per (b, c)
    xp = np.zeros((B, C, 18, 18, 18), np.float32)
    xp[:, :, 1:17, 1:17, 1:17] = x
    xp = xp.reshape(B, C, 18, ROW)

    in_maps = []
    for i in range(NCORES):
        xl = np.zeros((128, LOCVIEW), np.float32)
        for j in range(4):
            g = 2 * i - 1 + j  # global h-slab (padded index g+1)
            view = xp[:, :, g + 1].reshape(128, ROW)
            xl[:, LOCPAD + j * ROW : LOCPAD + (j + 1) * ROW] = view
        m = dict(shared)
        m["x_loc"] = xl.astype(ml_dtypes.bfloat16)
        in_maps.append(m)
    return in_maps


_PROG_CACHE = {}


def kernel(**inputs) -> np.ndarray:
    if "nc" not in _PROG_CACHE:
        _PROG_CACHE["nc"] = build_program()
    nc = _PROG_CACHE["nc"]
    in_maps = _prep_host(inputs)
    res = run_bass_kernel_spmd(nc, in_maps, list(range(NCORES))).results
    out = np.zeros((B, C, HH, HH, HH), np.float32)
    ov = out.reshape(B, C, 8, 2, SLAB)
    for i in range(NCORES):
        ov[:, :, i] = res[i]["out"].reshape(B, C, 2, SLAB)
    return out


if __name__ == "__main__":
    # smoke test with random data of the right shapes
    rng = np.random.default_rng(0)
    ins = {
        "x": rng.standard_normal((B, C, HH, HH, HH), dtype=np.float32),
    }
    print("building program...")
    nc = build_program()
    print("ok")



# revision 4
# speedup vs baseline: 1.3165x; 1.3165x over previous
"""Trainium2 Bass kernel for DAResBlock3D (dual-attention residual block).

Strategy (8 NeuronCores, SPMD):
  - Spatial sharding over H: core i owns output h-slabs {2i, 2i+1} (512 of
    4096 positions per batch), both batches on-chip as partition halves.
  - 3x3x3 convs: 27 shifted matmuls over a zero-padded local view (4 h-slabs
    with halo), with 2-way PE packing: row groups = batch.
  - BatchNorm (train-mode, global stats): per-core partial sums AllGathered
    (1KB) and reduced redundantly on every core.
  - PAM: energy computed transposed (E^T tiles, m on partitions); softmax
    without max-subtraction (energies are small); exp on ScalarE in
    (128,1024) chunks; O = v @ A^T via augmented v^T (ones column gives the
    softmax denominator for free).
  - CAM: per-core partial Gram (64x64) AllGathered; softmax redundant.
  - Cross-core data: AllGather collectives through DRAM bounce buffers.

Host side: the Bass program and its jitted PJRT executable are built once
per process and cached; per call only the input prep, transfer, execute and
output fetch are paid.  build_program(reps=K) emits the whole computation K
times back-to-back in one NEFF (shared tiles serialize the reps), which
test.py uses to measure the marginal per-execution hardware time.
"""

import os
import sys

sys.path.insert(0, "/opt/trn_rl_repo")

import numpy as np

import concourse.bass as bass
import concourse.mybir as mybir
import concourse.tile as tile
from concourse import bacc
from concourse.masks import make_identity

F32 = mybir.dt.float32
F32R = mybir.dt.float32r
BF16 = mybir.dt.bfloat16
U32 = mybir.dt.uint32
AF = mybir.ActivationFunctionType
ALU = mybir.AluOpType
AX = mybir.AxisListType

NCORES = 8
B = 2
C = 64
HH = 16
N = HH * HH * HH  # 4096
ROW = 18 * 18  # 324, one padded h-slab (w,d padded to 18x18)
LOCPAD = 19  # only w/d deltas (+-18, +-1) can underflow a slab base
LOCVIEW = LOCPAD + 4 * ROW + LOCPAD  # local act view: 4 h-slabs + margins
SLAB = 256  # interior positions per h-slab (16x16)
SHARD = 2 * SLAB  # 512 interior positions per batch per core
SLOPE = (1.0 / 8.0 + 1.0 / 3.0) / 2.0  # RReLU eval negative slope
EPS = 1e-5
NTOT = B * N  # BN normalization count = 8192

AG2_S1 = 2 * B * C * SLAB  # 65536: s1 region elems per rank
AG2_GRAM = B * C * C  # 8192: gram region elems per rank
AG2_PER = AG2_S1 + AG2_GRAM  # 73728


def _deltas():
    out = []
    for dh in (-1, 0, 1):
        for dw in (-1, 0, 1):
            for dd in (-1, 0, 1):
                out.append(dh * ROW + dw * 18 + dd)
    return out


DELTAS = _deltas()


def build_program(reps=1):
    nc = bacc.Bacc(
        "TRN2",
        target_bir_lowering=False,
        debug=False,
        num_devices=NCORES,
    )

    # ---- external inputs (per-core in_maps) ----
    x_loc = nc.dram_tensor("x_loc", [128, LOCVIEW], BF16, kind="ExternalInput")
    w_s = nc.dram_tensor("w_s", [128, 27, 64], BF16, kind="ExternalInput")
    w_c = nc.dram_tensor("w_c", [128, 27, 64], BF16, kind="ExternalInput")
    w_s1 = nc.dram_tensor("w_s1", [128, 27, 64], BF16, kind="ExternalInput")
    w_c1 = nc.dram_tensor("w_c1", [128, 27, 64], BF16, kind="ExternalInput")
    w_f = nc.dram_tensor("w_f", [128, 27, 64], BF16, kind="ExternalInput")
    qw_d = nc.dram_tensor("qw", [65, 64], BF16, kind="ExternalInput")
    kw_d = nc.dram_tensor("kw", [65, 64], BF16, kind="ExternalInput")
    vw_d = nc.dram_tensor("vw", [65, 66], BF16, kind="ExternalInput")
    bnp_d = nc.dram_tensor("bnp", [64, 10], F32, kind="ExternalInput")
    gam_d = nc.dram_tensor("gam", [1, 2], F32, kind="ExternalInput")
    out_d = nc.dram_tensor("out", [B, C, SHARD], F32, kind="ExternalOutput")

    rg = [list(range(NCORES))]

    with tile.TileContext(nc) as tc:
        dram_cm = tc.tile_pool(name="dram", bufs=1, space="DRAM")
        dram = dram_cm.__enter__()
        # collective bounce buffers (shared across reps; WAW deps serialize)
        st1_in = dram.tile([64, 4], F32)
        st1_out = dram.tile([NCORES, 64, 4], F32, addr_space="Shared")
        ag2_in = dram.tile([AG2_PER], F32)
        ag2_out = dram.tile([NCORES * AG2_PER], F32, addr_space="Shared")
        c2_in = dram.tile([2, B, C, SLAB], BF16)
        c2_ag = dram.tile([NCORES, 2, B, C, SLAB], BF16, addr_space="Shared")
        c2_ri = dram.tile([NCORES, 2, B, C, SLAB], BF16)
        c2_ro = dram.tile([2, B, C, SLAB], BF16)
        s2_in = dram.tile([2, B, C, SLAB], BF16)
        s2_ag = dram.tile([NCORES, 2, B, C, SLAB], BF16, addr_space="Shared")
        s2_ri = dram.tile([NCORES, 2, B, C, SLAB], BF16)
        s2_ro = dram.tile([2, B, C, SLAB], BF16)
        st2_in = dram.tile([64, 4], F32)
        st2_out = dram.tile([NCORES, 64, 4], F32, addr_space="Shared")
        fc_in = dram.tile([2, B, 2 * C, SLAB], BF16)
        fc_ag = dram.tile([NCORES, 2, B, 2 * C, SLAB], BF16, addr_space="Shared")
        fc_ri = dram.tile([NCORES, 2, B, 2 * C, SLAB], BF16)
        fc_ro = dram.tile([2, B, 2 * C, SLAB], BF16)
        stf_in = dram.tile([64, 2], F32)
        stf_out = dram.tile([NCORES, 64, 2], F32, addr_space="Shared")
        bcast_dram = dram.tile([B, SHARD], F32)

        singles_cm = tc.tile_pool(name="singles", bufs=1)
        singles = singles_cm.__enter__()

        ident = singles.tile([64, 64], BF16)
        make_identity(nc, ident[:])
        ident_f32 = singles.tile([64, 64], F32)
        make_identity(nc, ident_f32[:])

        # constants to SBUF
        qw_sb = singles.tile([65, 64], BF16)
        kw_sb = singles.tile([65, 64], BF16)
        vw_sb = singles.tile([65, 66], BF16)
        bnp = singles.tile([64, 10], F32)
        gam_p = singles.tile([1, 1], F32)
        gam_c_col = singles.tile([64, 1], F32)
        ones_row = singles.tile([1, 64], F32)
        nc.sync.dma_start(out=qw_sb[:], in_=qw_d[:])
        nc.sync.dma_start(out=kw_sb[:], in_=kw_d[:])
        nc.sync.dma_start(out=vw_sb[:], in_=vw_d[:])
        nc.sync.dma_start(out=bnp[:], in_=bnp_d[:])
        nc.sync.dma_start(out=gam_p[:], in_=gam_d[0:1, 0:1])
        nc.sync.dma_start(
            out=gam_c_col[:],
            in_=bass.AP(tensor=gam_d, offset=1, ap=[[0, 64], [1, 1]]),
        )
        nc.vector.memset(ones_row[:], 1.0)
        eps_col = singles.tile([64, 1], F32)
        nc.vector.memset(eps_col[:], EPS)
        zrow = singles.tile([128, SLAB], BF16)
        nc.vector.memset(zrow[:], 0.0)

        # big persistent activations
        acts_cm = tc.tile_pool(name="acts", bufs=1)
        acts = acts_cm.__enter__()
        x_sb = acts.tile([128, LOCVIEW], BF16)
        nc.sync.dma_start(out=x_sb[:], in_=x_loc[:])

        s1_own = [acts.tile([65, SHARD], F32, name=f"s1own{b}") for b in range(B)]
        s1_own_bf = [acts.tile([65, SHARD], BF16, name=f"s1ownbf{b}") for b in range(B)]
        c1_own = [acts.tile([64, SHARD], F32, name=f"c1own{b}") for b in range(B)]
        c1_own_bf = [acts.tile([64, SHARD], BF16, name=f"c1ownbf{b}") for b in range(B)]
        for b in range(B):
            nc.vector.memset(s1_own[b][64:65, :], 1.0)
            nc.vector.memset(s1_own_bf[b][64:65, :], 1.0)

        s1_pam = [acts.tile([65, N], BF16, name=f"s1pam{b}") for b in range(B)]
        for b in range(B):
            nc.vector.memset(s1_pam[b][64:65, :], 1.0)

        k_stack = acts.tile([128, N], BF16)
        q_stack = acts.tile([128, SHARD], BF16)
        vt_sb = [acts.tile([128, 32 * 66], BF16, name=f"vt{b}") for b in range(B)]

        # hoisted per-rep reusable activations (same name => same slot)
        c2both = acts.tile([128, SHARD], BF16)
        s2both = acts.tile([128, SHARD], BF16)
        c2_loc = acts.tile([128, LOCVIEW], BF16)
        s2_loc = acts.tile([128, LOCVIEW], BF16)
        fcat_own = acts.tile([128, B * SHARD], BF16)
        fcat_loc = [acts.tile([128, LOCVIEW], BF16, name=f"fl{b}") for b in range(B)]
        out_own = acts.tile([128, SHARD], F32)
        # zero the padded views once; reps only rewrite interiors/halos
        nc.gpsimd.memset(c2_loc[:], 0.0)
        nc.gpsimd.memset(s2_loc[:], 0.0)
        for b in range(B):
            nc.gpsimd.memset(fcat_loc[b][:], 0.0)

        wpool_cm = tc.tile_pool(name="wpool", bufs=2)
        wpool = wpool_cm.__enter__()

        stats_pool_cm = tc.tile_pool(name="stats", bufs=1)
        stats_pool = stats_pool_cm.__enter__()

        tmp_pool_cm = tc.tile_pool(name="tmp", bufs=2)
        tmp_pool = tmp_pool_cm.__enter__()

        # ---------------- helpers ----------------
        def load_wconv(dram_t, name):
            w = wpool.tile([128, 27, 64], BF16, tag="wconv", name=name)
            nc.sync.dma_start(out=w[:], in_=dram_t[:])
            return w

        def conv64(w_sb_t, act, psum_pool, tname):
            """3x3x3 conv over 64-ch padded local view for own 2 slabs.

            Returns per-batch compact raw-output tiles t[b] (64, 512) plus
            (sum, sumsq) stat columns (64,1) each."""
            touts = []
            stats = []
            for b in range(B):
                t = stats_pool.tile([64, SHARD], F32, name=f"{tname}_t{b}")
                for jj, jslab in enumerate((1, 2)):
                    ps = psum_pool.tile(
                        [64, ROW], F32, tag=f"convps{b}", name=f"{tname}ps{b}{jj}"
                    )
                    base = LOCPAD + jslab * ROW
                    for o in range(27):
                        nc.tensor.matmul(
                            ps[:],
                            lhsT=w_sb_t[64 * b : 64 * b + 64, o, :],
                            rhs=act[
                                64 * b : 64 * b + 64,
                                base + DELTAS[o] : base + DELTAS[o] + ROW,
                            ],
                            start=(o == 0),
                            stop=(o == 26),
                            tile_position=(64 * b, 0),
                        )
                    nc.vector.tensor_copy(
                        t[:, jj * SLAB : (jj + 1) * SLAB],
                        ps[:, :].rearrange("p (w d) -> p w d", w=18)[
                            :, 1:17, 1:17
                        ],
                    )
                touts.append(t)
                ssum = stats_pool.tile([64, 1], F32, name=f"{tname}_s{b}")
                ssq = stats_pool.tile([64, 1], F32, name=f"{tname}_q{b}")
                scr2 = tmp_pool.tile([64, SHARD], F32, tag="scrB", name=f"{tname}scrB{b}")
                nc.vector.reduce_sum(ssum[:], t[:], axis=AX.X)
                nc.scalar.activation(scr2[:], t[:], AF.Square, accum_out=ssq[:])
                stats.append((ssum, ssq))
            return touts, stats

        def conv128(w_sb_t, act_pair, psum_pool, tname):
            """3x3x3 conv with 128 input channels (fused concat), per batch."""
            touts = []
            stats = []
            for b in range(B):
                t = stats_pool.tile([64, SHARD], F32, name=f"{tname}_t{b}")
                for jj, jslab in enumerate((1, 2)):
                    ps = psum_pool.tile(
                        [64, ROW], F32, tag=f"convps{b}", name=f"{tname}ps{b}{jj}"
                    )
                    base = LOCPAD + jslab * ROW
                    for o in range(27):
                        nc.tensor.matmul(
                            ps[:],
                            lhsT=w_sb_t[:, o, :],
                            rhs=act_pair[b][
                                :, base + DELTAS[o] : base + DELTAS[o] + ROW
                            ],
                            start=(o == 0),
                            stop=(o == 26),
                        )
                    nc.vector.tensor_copy(
                        t[:, jj * SLAB : (jj + 1) * SLAB],
                        ps[:, :].rearrange("p (w d) -> p w d", w=18)[
                            :, 1:17, 1:17
                        ],
                    )
                touts.append(t)
                ssum = stats_pool.tile([64, 1], F32, name=f"{tname}_s{b}")
                ssq = stats_pool.tile([64, 1], F32, name=f"{tname}_q{b}")
                scr2 = tmp_pool.tile([64, SHARD], F32, tag="scrB", name=f"{tname}scrB{b}")
                nc.vector.reduce_sum(ssum[:], t[:], axis=AX.X)
                nc.scalar.activation(scr2[:], t[:], AF.Square, accum_out=ssq[:])
                stats.append((ssum, ssq))
            return touts, stats

        def pack_stats(dst_sb, stats_list):
            """stats_list: list of (ssum_b0, ssq_b0), (ssum_b1, ssq_b1) pairs
            per conv; writes [sum, sq] per conv into dst columns."""
            for ci, st in enumerate(stats_list):
                (s0, q0), (s1_, q1) = st
                nc.vector.tensor_add(dst_sb[:, 2 * ci : 2 * ci + 1], s0[:], s1_[:])
                nc.vector.tensor_add(
                    dst_sb[:, 2 * ci + 1 : 2 * ci + 2], q0[:], q1[:]
                )

        def bn_coeffs(tot_sb, col, g_col, b_col, name):
            """From total [sum, sumsq] cols compute A=(g*rstd), B=b-mean*A and
            the rrelu-scaled variants. Returns (A, B, As, Bs) (64,1) tiles."""
            mean = stats_pool.tile([64, 1], F32, name=f"{name}_mean")
            var = stats_pool.tile([64, 1], F32, name=f"{name}_var")
            a_t = stats_pool.tile([64, 1], F32, name=f"{name}_A")
            b_t = stats_pool.tile([64, 1], F32, name=f"{name}_B")
            as_t = stats_pool.tile([64, 1], F32, name=f"{name}_As")
            bs_t = stats_pool.tile([64, 1], F32, name=f"{name}_Bs")
            scr = stats_pool.tile([64, 1], F32, name=f"{name}_scr")
            nc.vector.tensor_scalar(
                mean[:], tot_sb[:, col : col + 1], 1.0 / NTOT, None, ALU.mult
            )
            nc.vector.tensor_scalar(
                var[:], tot_sb[:, col + 1 : col + 2], 1.0 / NTOT, None, ALU.mult
            )
            nc.vector.tensor_mul(scr[:], mean[:], mean[:])
            nc.vector.tensor_sub(var[:], var[:], scr[:])
            # rstd = exp(-0.5*ln(var+eps)); avoids the Sqrt table set
            nc.scalar.activation(scr[:], var[:], AF.Ln, bias=eps_col[:])
            nc.vector.tensor_scalar(scr[:], scr[:], -0.5, None, ALU.mult)
            nc.scalar.activation(scr[:], scr[:], AF.Exp)
            nc.vector.tensor_mul(a_t[:], scr[:], g_col)
            nc.vector.tensor_mul(scr[:], mean[:], a_t[:])
            nc.vector.tensor_sub(b_t[:], b_col, scr[:])
            nc.vector.tensor_scalar(as_t[:], a_t[:], SLOPE, None, ALU.mult)
            nc.vector.tensor_scalar(bs_t[:], b_t[:], SLOPE, None, ALU.mult)
            return a_t, b_t, as_t, bs_t

        def bn_rrelu(t_raw, coeffs, dst_ap):
            """dst = max(A*t+B, As*t+Bs) elementwise."""
            a_t, b_t, as_t, bs_t = coeffs
            y1 = tmp_pool.tile([64, SHARD], F32, tag="y1", name="y1_t")
            y2 = tmp_pool.tile([64, SHARD], F32, tag="y2", name="y2_t")
            nc.vector.tensor_scalar(
                y1[:], t_raw[:], a_t[:], b_t[:], ALU.mult, ALU.add
            )
            nc.vector.tensor_scalar(
                y2[:], t_raw[:], as_t[:], bs_t[:], ALU.mult, ALU.add
            )
            nc.vector.tensor_max(dst_ap, y1[:], y2[:])

        def halo_exchange(in_t, ag_t, ri_t, ro_t, nch):
            """AG own slabs, then RS-rotate so each core receives exactly its
            lo/hi halo slabs (slot-static reads of the gathered buffer)."""
            nc.gpsimd.collective_compute(
                "AllGather", ALU.bypass, replica_groups=rg,
                ins=[in_t[:].opt()], outs=[ag_t[:].opt()],
            )
            blk = B * nch * SLAB  # one slab block (elements)
            per = 2 * blk  # one rank contribution
            for i in range(NCORES):
                # lo slot: rank i-1's slab 1
                if i > 0:
                    nc.sync.dma_start(
                        out=bass.AP(
                            tensor=ri_t[:].tensor,
                            offset=i * per,
                            ap=[[1, blk]],
                        ),
                        in_=bass.AP(
                            tensor=ag_t[:].tensor,
                            offset=(i - 1) * per + blk,
                            ap=[[1, blk]],
                        ),
                    )
                else:
                    for z in range(blk // (128 * SLAB)):
                        nc.sync.dma_start(
                            out=bass.AP(
                                tensor=ri_t[:].tensor,
                                offset=z * 128 * SLAB,
                                ap=[[SLAB, 128], [1, SLAB]],
                            ),
                            in_=zrow[:],
                        )
                # hi slot: rank i+1's slab 0
                if i < NCORES - 1:
                    nc.sync.dma_start(
                        out=bass.AP(
                            tensor=ri_t[:].tensor,
                            offset=i * per + blk,
                            ap=[[1, blk]],
                        ),
                        in_=bass.AP(
                            tensor=ag_t[:].tensor,
                            offset=(i + 1) * per,
                            ap=[[1, blk]],
                        ),
                    )
                else:
                    for z in range(blk // (128 * SLAB)):
                        nc.sync.dma_start(
                            out=bass.AP(
                                tensor=ri_t[:].tensor,
                                offset=i * per + blk + z * 128 * SLAB,
                                ap=[[SLAB, 128], [1, SLAB]],
                            ),
                            in_=zrow[:],
                        )
            nc.gpsimd.collective_compute(
                "ReduceScatter", ALU.add, replica_groups=rg,
                ins=[ri_t[:].opt()], outs=[ro_t[:].opt()],
            )

        def build_view(ro_t, nch, bsel, dst, own_ap, name):
            """dst (128, LOCVIEW) bf16: slabs 1-2 <- own; 0/3 <- RS halos/8."""
            blk = B * nch * SLAB
            boff = 0 if bsel is None else bsel * nch * SLAB
            for dslab, hs in ((0, 0), (3, 1)):
                stg = tmp_pool.tile(
                    [128, SLAB], BF16, tag="hstg", name=f"hs{name}{dslab}"
                )
                nc.sync.dma_start(
                    out=stg[:],
                    in_=bass.AP(
                        tensor=ro_t[:].tensor,
                        offset=hs * blk + boff,
                        ap=[[SLAB, 128], [1, SLAB]],
                    ),
                )
                nc.vector.tensor_scalar(
                    dst[:, LOCPAD + dslab * ROW : LOCPAD + (dslab + 1) * ROW]
                    .rearrange("p (w d) -> p w d", w=18)[:, 1:17, 1:17],
                    stg[:].rearrange("p (w d) -> p w d", w=16),
                    1.0 / NCORES,
                    None,
                    ALU.mult,
                )
            nc.vector.tensor_copy(
                dst[:, LOCPAD + 1 * ROW : LOCPAD + 3 * ROW]
                .rearrange("p (j w d) -> p j w d", j=2, w=18)[:, :, 1:17, 1:17],
                own_ap,
            )

        def emit_body(rep):
            # =========== phase 1: conv S and conv C (input x) ===========
            cpsum_cm = tc.tile_pool(name=f"cpsum_r{rep}", bufs=2, space="PSUM")
            cpsum = cpsum_cm.__enter__()

            ws_sb = load_wconv(w_s, "wsS")
            tS, statS = conv64(ws_sb, x_sb, cpsum, "cS")
            wc_sb = load_wconv(w_c, "wsC")
            tC, statC = conv64(wc_sb, x_sb, cpsum, "cC")

            st1_sb = stats_pool.tile([64, 4], F32, name="st1_sb")
            pack_stats(st1_sb, [statS, statC])
            nc.sync.dma_start(out=st1_in[:], in_=st1_sb[:])
            nc.gpsimd.collective_compute(
                "AllGather",
                ALU.bypass,
                replica_groups=rg,
                ins=[st1_in[:].opt()],
                outs=[st1_out[:].opt()],
            )

            # reduce gathered stats and compute BN coefficients
            st1_stage = stats_pool.tile([64, 4, NCORES], F32, name="st1_stage")
            nc.sync.dma_start(
                out=st1_stage[:],
                in_=bass.AP(
                    tensor=st1_out[:].tensor,
                    offset=0,
                    ap=[[4, 64], [1, 4], [256, NCORES]],
                ),
            )
            st1_tot = stats_pool.tile([64, 4], F32, name="st1_tot")
            nc.vector.tensor_reduce(st1_tot[:], st1_stage[:], axis=AX.X, op=ALU.add)
            cS = bn_coeffs(st1_tot, 0, bnp[:, 0:1], bnp[:, 1:2], "bnS")
            cC = bn_coeffs(st1_tot, 2, bnp[:, 2:3], bnp[:, 3:4], "bnC")

            for b in range(B):
                bn_rrelu(tS[b], cS, s1_own[b][0:64, :])
                bn_rrelu(tC[b], cC, c1_own[b][:, :])
                nc.vector.tensor_copy(s1_own_bf[b][0:64, :], s1_own[b][0:64, :])
                nc.vector.tensor_copy(c1_own_bf[b][:, :], c1_own[b][:, :])

            cpsum_cm.__exit__(None, None, None)

            # =========== phase 2: CAM partial gram + AG2 (s1 + gram) ===========
            mpsum_cm = tc.tile_pool(name=f"mpsum_r{rep}", bufs=2, space="PSUM")
            mpsum = mpsum_cm.__enter__()

            ft_sb = [tmp_pool.tile([128, 4 * 64], BF16, tag=f"ft{b}", name=f"ft{b}") for b in range(B)]
            gram_sb = tmp_pool.tile([64, B * 64], F32, tag="gram", name="gram_sb")
            for b in range(B):
                for kk in range(4):
                    pst = mpsum.tile([128, 64], BF16, tag="mm", name=f"ft{b}{kk}")
                    nc.tensor.transpose(
                        pst[:],
                        c1_own_bf[b][:, 128 * kk : 128 * (kk + 1)],
                        ident[:],
                    )
                    nc.vector.tensor_copy(
                        ft_sb[b][:, 64 * kk : 64 * (kk + 1)], pst[:, 0:64]
                    )
                psg = mpsum.tile([64, 64], F32, tag="mm", name=f"gram{b}")
                for kk in range(4):
                    nc.tensor.matmul(
                        psg[:],
                        lhsT=ft_sb[b][:, 64 * kk : 64 * (kk + 1)],
                        rhs=ft_sb[b][:, 64 * kk : 64 * (kk + 1)],
                        start=(kk == 0),
                        stop=(kk == 3),
                    )
                nc.vector.tensor_copy(gram_sb[:, 64 * b : 64 * (b + 1)], psg[:])

            # write AG2 contribution: s1 (slab-major) + gram
            for b in range(B):
                nc.sync.dma_start(
                    out=bass.AP(
                        tensor=ag2_in[:].tensor,
                        offset=b * C * SLAB,
                        ap=[[SLAB, 64], [B * C * SLAB, 2], [1, SLAB]],
                    ),
                    in_=s1_own[b][0:64, :].rearrange("p (j s) -> p j s", j=2),
                )
            nc.sync.dma_start(
                out=bass.AP(
                    tensor=ag2_in[:].tensor,
                    offset=AG2_S1,
                    ap=[[64, 64], [64 * 64, B], [1, 64]],
                ),
                in_=gram_sb[:].rearrange("p (b c) -> p b c", b=B),
            )
            nc.gpsimd.collective_compute(
                "AllGather",
                ALU.bypass,
                replica_groups=rg,
                ins=[ag2_in[:].opt()],
                outs=[ag2_out[:].opt()],
            )

            # =========== phase 3: q (local), then k/vT from gathered s1 ===========
            for b in range(B):
                psq = mpsum.tile([64, SHARD], F32, tag="qk", name=f"q{b}")
                nc.tensor.matmul(
                    psq[:],
                    lhsT=qw_sb[:],
                    rhs=s1_own_bf[b][:],
                    start=True,
                    stop=True,
                )
                nc.vector.tensor_copy(q_stack[64 * b : 64 * (b + 1), :], psq[:])

            # load gathered s1 into s1_pam (global n order); one DMA per slab half
            for b in range(B):
                for j in range(2):
                    nc.gpsimd.dma_start(
                        out=s1_pam[b][0:64, :]
                        .rearrange("p (g s) -> p g s", s=2 * SLAB)[:, :, j * SLAB : (j + 1) * SLAB],
                        in_=bass.AP(
                            tensor=ag2_out[:].tensor,
                            offset=b * C * SLAB + j * B * C * SLAB,
                            ap=[[SLAB, 64], [AG2_PER, NCORES], [1, SLAB]],
                        ),
                    )
            # gathered gram -> reduce over cores
            gram_full = [tmp_pool.tile([64, 64], F32, tag=f"gramf{b}", name=f"gramf{b}") for b in range(B)]
            for b in range(B):
                gstage = tmp_pool.tile(
                    [64, 64, NCORES], F32, tag="gstage", name=f"gstage{b}"
                )
                nc.sync.dma_start(
                    out=gstage[:],
                    in_=bass.AP(
                        tensor=ag2_out[:].tensor,
                        offset=AG2_S1 + b * C * C,
                        ap=[[64, 64], [1, 64], [AG2_PER, NCORES]],
                    ),
                )
                nc.vector.tensor_reduce(gram_full[b][:], gstage[:], axis=AX.X, op=ALU.add)

            for b in range(B):
                for nt in range(8):
                    psk = mpsum.tile([64, 512], F32, tag="qk", name=f"k{b}{nt}")
                    nc.tensor.matmul(
                        psk[:],
                        lhsT=kw_sb[:],
                        rhs=s1_pam[b][:, 512 * nt : 512 * (nt + 1)],
                        start=True,
                        stop=True,
                    )
                    nc.vector.tensor_copy(
                        k_stack[64 * b : 64 * (b + 1), 512 * nt : 512 * (nt + 1)],
                        psk[:],
                    )
                for mt in range(32):
                    psv = mpsum.tile([128, 66], F32, tag="vt", name=f"v{b}{mt}")
                    nc.tensor.matmul(
                        psv[:],
                        lhsT=s1_pam[b][:, 128 * mt : 128 * (mt + 1)],
                        rhs=vw_sb[:],
                        start=True,
                        stop=True,
                    )
                    nc.vector.tensor_copy(
                        vt_sb[b][:, 66 * mt : 66 * (mt + 1)], psv[:]
                    )

            # =========== phase 4: CAM finish -> c2 -> pair halo AG ===========
            for b in range(B):
                rowmax = tmp_pool.tile([64, 1], F32, tag="camx", name=f"camx{b}")
                den = tmp_pool.tile([64, 1], F32, tag="camd", name=f"camd{b}")
                attn = tmp_pool.tile([64, 64], F32, tag="cama", name=f"cama{b}")
                nc.vector.tensor_reduce(
                    rowmax[:], gram_full[b][:], axis=AX.X, op=ALU.min
                )
                nc.scalar.activation(
                    attn[:],
                    gram_full[b][:],
                    AF.Exp,
                    bias=rowmax[:],
                    scale=-1.0,
                    accum_out=den[:],
                )
                nc.vector.reciprocal(den[:], den[:])
                nc.vector.tensor_scalar(attn[:], attn[:], den[:], None, ALU.mult)
                # attn^T via PE
                psat = mpsum.tile([64, 64], F32, tag="mm", name=f"at{b}")
                nc.tensor.transpose(psat[:], attn[:], ident_f32[:])
                attnT = tmp_pool.tile([64, 64], BF16, tag="camat", name=f"camat{b}")
                nc.vector.tensor_copy(attnT[:], psat[:])
                # cam_out = attnT.T @ c1_own
                psco = mpsum.tile([64, SHARD], F32, tag="qk", name=f"co{b}")
                nc.tensor.matmul(
                    psco[:],
                    lhsT=attnT[:],
                    rhs=c1_own_bf[b][:],
                    start=True,
                    stop=True,
                )
                c2t = tmp_pool.tile([64, SHARD], F32, tag="c2t", name=f"c2t{b}")
                nc.vector.tensor_scalar(c2t[:], psco[:], gam_c_col[:, 0:1], None, ALU.mult)
                nc.vector.tensor_add(
                    c2both[64 * b : 64 * (b + 1), :], c2t[:], c1_own[b][:]
                )
                nc.sync.dma_start(
                    out=bass.AP(
                        tensor=c2_in[:].tensor,
                        offset=b * C * SLAB,
                        ap=[[SLAB, 64], [B * C * SLAB, 2], [1, SLAB]],
                    ),
                    in_=c2both[64 * b : 64 * (b + 1), :].rearrange(
                        "p (j s) -> p j s", j=2
                    ),
                )
            halo_exchange(c2_in, c2_ag, c2_ri, c2_ro, C)

            mpsum_cm.__exit__(None, None, None)

            # =========== phase 5: PAM attention ===========
            epsum_cm = tc.tile_pool(name=f"epsum_r{rep}", bufs=3, space="PSUM")
            epsum = epsum_cm.__enter__()
            opsum_cm = tc.tile_pool(name=f"opsum_r{rep}", bufs=1, space="PSUM")
            opsum = opsum_cm.__enter__()
            apool_cm = tc.tile_pool(name=f"apool_r{rep}", bufs=3)
            apool = apool_cm.__enter__()

            o_ps = [
                opsum.tile([65, SHARD], F32, name=f"ops{b}", tag=f"ops{b}")
                for b in range(B)
            ]
            for g2 in range(16):
                for b in range(B):
                    e_ps = epsum.tile([128, 1024], F32, tag="eg", name=f"e{g2}{b}")
                    for j in range(2):
                        mt = 2 * g2 + j
                        nc.tensor.matmul(
                            e_ps[:, 512 * j : 512 * (j + 1)],
                            lhsT=k_stack[
                                64 * b : 64 * (b + 1), 128 * mt : 128 * (mt + 1)
                            ],
                            rhs=q_stack[64 * b : 64 * (b + 1), :],
                            start=True,
                            stop=True,
                            tile_position=(64 * b, 0),
                        )
                    a_sb = apool.tile([128, 1024], BF16, tag="ag", name=f"a{g2}{b}")
                    nc.scalar.activation(a_sb[:], e_ps[:], AF.Exp)
                    for j in range(2):
                        mt = 2 * g2 + j
                        nc.tensor.matmul(
                            o_ps[b][:],
                            lhsT=vt_sb[b][:, 66 * mt : 66 * mt + 65],
                            rhs=a_sb[:, 512 * j : 512 * (j + 1)],
                            start=(mt == 0),
                            stop=(mt == 31),
                        )

            # =========== phase 6: PAM finalize -> s2 -> pair halo AG ===========
            for b in range(B):
                recip = tmp_pool.tile([1, SHARD], F32, tag="rec", name=f"rec{b}")
                recipg = tmp_pool.tile([1, SHARD], F32, tag="recg", name=f"recg{b}")
                nc.vector.reciprocal(recip[:], o_ps[b][64:65, :])
                nc.vector.tensor_scalar(
                    recipg[:], recip[:], gam_p[:, 0:1], None, ALU.mult
                )
                nc.sync.dma_start(out=bcast_dram[b : b + 1, :], in_=recipg[:])
                bc_sb = tmp_pool.tile([64, SHARD], F32, tag="bcs", name=f"bcs{b}")
                nc.sync.dma_start(
                    out=bc_sb[:],
                    in_=bass.AP(
                        tensor=bcast_dram[:].tensor,
                        offset=b * SHARD,
                        ap=[[0, 64], [1, SHARD]],
                    ),
                )
                s2t = tmp_pool.tile([64, SHARD], F32, tag="s2t", name=f"s2t{b}")
                nc.vector.tensor_mul(s2t[:], o_ps[b][0:64, :], bc_sb[:])
                nc.vector.tensor_add(
                    s2both[64 * b : 64 * (b + 1), :], s2t[:], s1_own[b][0:64, :]
                )
                nc.sync.dma_start(
                    out=bass.AP(
                        tensor=s2_in[:].tensor,
                        offset=b * C * SLAB,
                        ap=[[SLAB, 64], [B * C * SLAB, 2], [1, SLAB]],
                    ),
                    in_=s2both[64 * b : 64 * (b + 1), :].rearrange(
                        "p (j s) -> p j s", j=2
                    ),
                )
            halo_exchange(s2_in, s2_ag, s2_ri, s2_ro, C)

            for p in (apool_cm, opsum_cm, epsum_cm):
                p.__exit__(None, None, None)
            cpsum_cm = tc.tile_pool(name=f"cpsum2_r{rep}", bufs=2, space="PSUM")
            cpsum = cpsum_cm.__enter__()

            # =========== phase 7: conv C1 (on gathered c2) ===========
            build_view(
                c2_ro, C, None, c2_loc,
                c2both[:].rearrange("p (j w d) -> p j w d", j=2, w=16), "c2",
            )
            wc1_sb = load_wconv(w_c1, "wsC1")
            tC1, statC1 = conv64(wc1_sb, c2_loc, cpsum, "cC1")

            # =========== phase 8: conv S1 (on gathered s2) ===========
            build_view(
                s2_ro, C, None, s2_loc,
                s2both[:].rearrange("p (j w d) -> p j w d", j=2, w=16), "s2",
            )
            ws1_sb = load_wconv(w_s1, "wsS1")
            tS1, statS1 = conv64(ws1_sb, s2_loc, cpsum, "cS1")

            st2_sb = stats_pool.tile([64, 4], F32, name="st2_sb")
            pack_stats(st2_sb, [statS1, statC1])
            nc.sync.dma_start(out=st2_in[:], in_=st2_sb[:])
            nc.gpsimd.collective_compute(
                "AllGather",
                ALU.bypass,
                replica_groups=rg,
                ins=[st2_in[:].opt()],
                outs=[st2_out[:].opt()],
            )
            st2_stage = stats_pool.tile([64, 4, NCORES], F32, name="st2_stage")
            nc.sync.dma_start(
                out=st2_stage[:],
                in_=bass.AP(
                    tensor=st2_out[:].tensor,
                    offset=0,
                    ap=[[4, 64], [1, 4], [256, NCORES]],
                ),
            )
            st2_tot = stats_pool.tile([64, 4], F32, name="st2_tot")
            nc.vector.tensor_reduce(st2_tot[:], st2_stage[:], axis=AX.X, op=ALU.add)
            cS1 = bn_coeffs(st2_tot, 0, bnp[:, 4:5], bnp[:, 5:6], "bnS1")
            cC1 = bn_coeffs(st2_tot, 2, bnp[:, 6:7], bnp[:, 7:8], "bnC1")

            for b in range(B):
                bn_rrelu(tS1[b], cS1, fcat_own[0:64, b * SHARD : (b + 1) * SHARD])
                bn_rrelu(tC1[b], cC1, fcat_own[64:128, b * SHARD : (b + 1) * SHARD])

            # contribution: (2 slab, 2 b, 128 c, 256); one DMA per batch
            for b in range(B):
                nc.sync.dma_start(
                    out=bass.AP(
                        tensor=fc_in[:].tensor,
                        offset=b * 2 * C * SLAB,
                        ap=[[SLAB, 128], [B * 2 * C * SLAB, 2], [1, SLAB]],
                    ),
                    in_=fcat_own[:, b * SHARD : (b + 1) * SHARD].rearrange(
                        "p (j s) -> p j s", j=2
                    ),
                )
            halo_exchange(fc_in, fc_ag, fc_ri, fc_ro, 2 * C)

            # =========== phase 9: conv F ===========
            for b in range(B):
                build_view(
                    fc_ro, 2 * C, b, fcat_loc[b],
                    fcat_own[:, b * SHARD : (b + 1) * SHARD].rearrange(
                        "p (j w d) -> p j w d", j=2, w=16
                    ),
                    f"fc{b}",
                )
            wf_sb = load_wconv(w_f, "wsF")
            tF, statF = conv128(wf_sb, fcat_loc, cpsum, "cF")

            stf_sb = stats_pool.tile([64, 2], F32, name="stf_sb")
            pack_stats(stf_sb, [statF])
            nc.sync.dma_start(out=stf_in[:], in_=stf_sb[:])
            nc.gpsimd.collective_compute(
                "AllGather",
                ALU.bypass,
                replica_groups=rg,
                ins=[stf_in[:].opt()],
                outs=[stf_out[:].opt()],
            )
            stf_stage = stats_pool.tile([64, 2, NCORES], F32, name="stf_stage")
            nc.sync.dma_start(
                out=stf_stage[:],
                in_=bass.AP(
                    tensor=stf_out[:].tensor,
                    offset=0,
                    ap=[[2, 64], [1, 2], [128, NCORES]],
                ),
            )
            stf_tot = stats_pool.tile([64, 2], F32, name="stf_tot")
            nc.vector.tensor_reduce(stf_tot[:], stf_stage[:], axis=AX.X, op=ALU.add)
            cF = bn_coeffs(stf_tot, 0, bnp[:, 8:9], bnp[:, 9:10], "bnF")

            for b in range(B):
                bn_rrelu(tF[b], cF, out_own[64 * b : 64 * (b + 1), :])
            nc.sync.dma_start(
                out=bass.AP(
                    tensor=out_d,
                    offset=0,
                    ap=[[SHARD, 128], [1, SHARD]],
                ),
                in_=out_own[:],
            )
            cpsum_cm.__exit__(None, None, None)

        for rep in range(reps):
            emit_body(rep)

        for p in (tmp_pool_cm, stats_pool_cm, wpool_cm, acts_cm,
                  singles_cm, dram_cm):
            p.__exit__(None, None, None)

    nc.finalize()
    return nc


def _prep_host(inputs):
    """Build per-core in_maps from the full problem inputs."""
    x = np.asarray(inputs["x"], np.float32)

    import ml_dtypes

    def conv_wT(w):
        # w: (O, I, 3, 3, 3) -> (128, 27, 64): [dup*64+i, off, o]
        wt = np.transpose(np.asarray(w, np.float32), (1, 2, 3, 4, 0)).reshape(
            w.shape[1], 27, 64
        )
        if w.shape[1] == 64:
            wt = np.concatenate([wt, wt], axis=0)
        return wt.astype(ml_dtypes.bfloat16)

    qw = np.asarray(inputs["qw"], np.float32).reshape(64, 64)
    kw = np.asarray(inputs["kw"], np.float32).reshape(64, 64)
    vw = np.asarray(inputs["vw"], np.float32).reshape(64, 64)
    qa = np.zeros((65, 64), np.float32)
    qa[:64] = qw.T
    qa[64] = np.asarray(inputs["qb"], np.float32)
    ka = np.zeros((65, 64), np.float32)
    ka[:64] = kw.T
    ka[64] = np.asarray(inputs["kb"], np.float32)
    va = np.zeros((65, 66), np.float32)
    va[:64, :64] = vw.T
    va[64, :64] = np.asarray(inputs["vb"], np.float32)
    va[64, 64] = 1.0

    bnp = np.stack(
        [
            np.asarray(inputs[k], np.float32)
            for k in ("gS", "bS", "gC", "bC", "gS1", "bS1", "gC1", "bC1", "gF", "bF")
        ],
        axis=1,
    )
    gam = np.array(
        [[float(np.asarray(inputs["gamma_p"]).reshape(-1)[0]),
          float(np.asarray(inputs["gamma_c"]).reshape(-1)[0])]],
        np.float32,
    )

    shared = {
        "w_s": conv_wT(inputs["wS"]),
        "w_c": conv_wT(inputs["wC"]),
        "w_s1": conv_wT(inputs["wS1"]),
        "w_c1": conv_wT(inputs["wC1"]),
        "w_f": conv_wT(inputs["wF"]),
        "qw": qa.astype(ml_dtypes.bfloat16),
        "kw": ka.astype(ml_dtypes.bfloat16),
        "vw": va.astype(ml_dtypes.bfloat16),
        "bnp": np.ascontiguousarray(bnp),
        "gam": gam,
    }

    # padded full volume (h, w, d) -> (18, 18, 18), flattened per (b, c)
    xp = np.zeros((B, C, 18, 18, 18), np.float32)
    xp[:, :, 1:17, 1:17, 1:17] = x
    xp = xp.reshape(B, C, 18, ROW)

    in_maps = []
    for i in range(NCORES):
        xl = np.zeros((128, LOCVIEW), np.float32)
        for j in range(4):
            g = 2 * i - 1 + j  # global h-slab (padded index g+1)
            view = xp[:, :, g + 1].reshape(128, ROW)
            xl[:, LOCPAD + j * ROW : LOCPAD + (j + 1) * ROW] = view
        m = dict(shared)
        m["x_loc"] = xl.astype(ml_dtypes.bfloat16)
        in_maps.append(m)
    return in_maps


class _Exec:
    """Compile-once executor: bass program + cached jitted PJRT callable."""

    def __init__(self, reps):
        import jax
        from jax.sharding import Mesh, PartitionSpec, NamedSharding
        from jax.experimental.shard_map import shard_map
        from concourse.bass2jax import (
            _bass_exec_p,
            install_neuronx_cc_hook,
            partition_id_tensor,
        )

        self.jax = jax
        self.nc = build_program(reps)
        install_neuronx_cc_hook()
        nc_ = self.nc
        partition_name = (
            nc_.partition_id_tensor.name if nc_.partition_id_tensor else None
        )
        in_names, out_names, out_avals, zero_shapes = [], [], [], []
        for alloc in nc_.m.functions[0].allocations:
            if not isinstance(alloc, mybir.MemoryLocationSet):
                continue
            name = alloc.memorylocations[0].name
            if alloc.kind == "ExternalInput":
                if name != partition_name:
                    in_names.append(name)
            elif alloc.kind == "ExternalOutput":
                shape = tuple(alloc.tensor_shape)
                dtype = mybir.dt.np(alloc.dtype)
                out_names.append(name)
                out_avals.append(jax.core.ShapedArray(shape, dtype))
                zero_shapes.append((shape, dtype))
        self.in_names = in_names
        self.out_names = out_names
        self.out_avals = out_avals
        self.zero_shapes = zero_shapes
        n_params = len(in_names)
        n_outs = len(out_avals)
        self.n_params = n_params
        all_in = in_names + out_names + ([partition_name] if partition_name else [])

        def _body(*args):
            operands = list(args)
            if partition_name:
                operands.append(partition_id_tensor())
            return tuple(
                _bass_exec_p.bind(
                    *operands,
                    out_avals=tuple(out_avals),
                    in_names=tuple(all_in),
                    out_names=tuple(out_names),
                    lowering_input_output_aliases=(),
                    sim_require_finite=True,
                    sim_require_nnan=True,
                    nc=nc_,
                )
            )

        devices = jax.devices()[:NCORES]
        assert len(devices) == NCORES
        self.mesh = Mesh(np.asarray(devices), ("core",))
        self.psharded = NamedSharding(self.mesh, PartitionSpec("core"))
        self.sharded = jax.jit(
            shard_map(
                _body,
                mesh=self.mesh,
                in_specs=(PartitionSpec("core"),) * (n_params + n_outs),
                out_specs=(PartitionSpec("core"),) * n_outs,
                check_rep=False,
            ),
            donate_argnums=tuple(range(n_params, n_params + n_outs)),
            keep_unused=True,
        )

    def concat_inputs(self, in_maps):
        return [
            np.concatenate([np.asarray(m[name]) for m in in_maps], axis=0)
            for name in self.in_names
        ]

    def zeros(self):
        return [
            np.zeros((NCORES * s[0], *s[1:]), d) for (s, d) in self.zero_shapes
        ]

    def run(self, in_maps):
        """Full path: numpy in -> per-core dict of numpy outputs."""
        concat_in = self.concat_inputs(in_maps)
        out_arrs = self.sharded(*concat_in, *self.zeros())
        return [
            {
                name: np.asarray(out_arrs[i]).reshape(
                    NCORES, *self.out_avals[i].shape
                )[c]
                for i, name in enumerate(self.out_names)
            }
            for c in range(NCORES)
        ]


_EXECS = {}


def _get_exec(reps=1):
    if reps not in _EXECS:
        _EXECS[reps] = _Exec(reps)
    return _EXECS[reps]


def _run_fallback(inputs):
    """Stock path for non-axon environments."""
    from concourse.bass_utils import run_bass_kernel_spmd

    if "nc" not in _EXECS:
        _EXECS["nc"] = build_program(1)
    nc_ = _EXECS["nc"]
    in_maps = _prep_host(inputs)
    return run_bass_kernel_spmd(nc_, in_maps, list(range(NCORES))).results


def kernel(**inputs) -> np.ndarray:
    from concourse._compat import axon_active

    if axon_active():
        ex = _get_exec(1)
        res = ex.run(_prep_host(inputs))
    else:
        res = _run_fallback(inputs)
    out = np.zeros((B, C, HH, HH, HH), np.float32)
    ov = out.reshape(B, C, 8, 2, SLAB)
    for i in range(NCORES):
        ov[:, :, i] = res[i]["out"].reshape(B, C, 2, SLAB)
    return out


if __name__ == "__main__":
    rng = np.random.default_rng(0)
    print("building program...")
    nc = build_program()
    print("ok")


# revision 6
# speedup vs baseline: 2591.7914x; 1968.6852x over previous
"""Trainium2 Bass kernel for DAResBlock3D (dual-attention residual block).

Strategy (8 NeuronCores, SPMD):
  - Spatial sharding over H: core i owns output h-slabs {2i, 2i+1} (512 of
    4096 positions per batch), both batches on-chip as partition halves.
  - 3x3x3 convs: 27 shifted matmuls over a zero-padded local view (4 h-slabs
    with halo), with 2-way PE packing: row groups = batch.
  - BatchNorm (train-mode, global stats): per-core partial sums AllGathered
    (1KB) and reduced redundantly on every core.
  - PAM: energy computed transposed (E^T tiles, m on partitions); softmax
    without max-subtraction (energies are small); exp on ScalarE in
    (128,1024) chunks; O = v @ A^T via augmented v^T (ones column gives the
    softmax denominator for free).
  - CAM: per-core partial Gram (64x64) AllGathered; softmax redundant.
  - Cross-core data: AllGather collectives through DRAM bounce buffers.

Host side: the Bass program and its jitted PJRT executable are built once
per process and cached; per call only the input prep, transfer, execute and
output fetch are paid.  build_program(reps=K) emits the whole computation K
times back-to-back in one NEFF (shared tiles serialize the reps), which
test.py uses to measure the marginal per-execution hardware time.
"""

import os
import sys

sys.path.insert(0, "/opt/trn_rl_repo")

import numpy as np

import concourse.bass as bass
import concourse.mybir as mybir
import concourse.tile as tile
from concourse import bacc
from concourse.masks import make_identity

F32 = mybir.dt.float32
F32R = mybir.dt.float32r
BF16 = mybir.dt.bfloat16
U32 = mybir.dt.uint32
AF = mybir.ActivationFunctionType
ALU = mybir.AluOpType
AX = mybir.AxisListType

NCORES = 8
B = 2
C = 64
HH = 16
N = HH * HH * HH  # 4096
ROW = 18 * 18  # 324, one padded h-slab (w,d padded to 18x18)
LOCPAD = 19  # only w/d deltas (+-18, +-1) can underflow a slab base
LOCVIEW = LOCPAD + 4 * ROW + LOCPAD  # local act view: 4 h-slabs + margins
SLAB = 256  # interior positions per h-slab (16x16)
SHARD = 2 * SLAB  # 512 interior positions per batch per core
SLOPE = (1.0 / 8.0 + 1.0 / 3.0) / 2.0  # RReLU eval negative slope
EPS = 1e-5
NTOT = B * N  # BN normalization count = 8192

AG2_S1 = 2 * B * C * SLAB  # 65536: s1 region elems per rank
AG2_GRAM = B * C * C  # 8192: gram region elems per rank
AG2_PER = AG2_S1 + AG2_GRAM  # 73728


def _deltas():
    out = []
    for dh in (-1, 0, 1):
        for dw in (-1, 0, 1):
            for dd in (-1, 0, 1):
                out.append(dh * ROW + dw * 18 + dd)
    return out


DELTAS = _deltas()


def build_program(reps=1):
    nc = bacc.Bacc(
        "TRN2",
        target_bir_lowering=False,
        debug=False,
        num_devices=NCORES,
    )

    # ---- external inputs (per-core in_maps) ----
    x_loc = nc.dram_tensor("x_loc", [128, LOCVIEW], BF16, kind="ExternalInput")
    w_s = nc.dram_tensor("w_s", [128, 27, 64], BF16, kind="ExternalInput")
    w_c = nc.dram_tensor("w_c", [128, 27, 64], BF16, kind="ExternalInput")
    w_s1 = nc.dram_tensor("w_s1", [128, 27, 64], BF16, kind="ExternalInput")
    w_c1 = nc.dram_tensor("w_c1", [128, 27, 64], BF16, kind="ExternalInput")
    w_f = nc.dram_tensor("w_f", [128, 27, 64], BF16, kind="ExternalInput")
    qw_d = nc.dram_tensor("qw", [65, 64], BF16, kind="ExternalInput")
    kw_d = nc.dram_tensor("kw", [65, 64], BF16, kind="ExternalInput")
    vw_d = nc.dram_tensor("vw", [65, 66], BF16, kind="ExternalInput")
    bnp_d = nc.dram_tensor("bnp", [64, 10], F32, kind="ExternalInput")
    gam_d = nc.dram_tensor("gam", [1, 2], F32, kind="ExternalInput")
    out_d = nc.dram_tensor("out", [B, C, SHARD], F32, kind="ExternalOutput")

    rg = [list(range(NCORES))]

    with tile.TileContext(nc) as tc:
        dram_cm = tc.tile_pool(name="dram", bufs=1, space="DRAM")
        dram = dram_cm.__enter__()
        # collective bounce buffers. Inputs/locals are shared across reps
        # (WAW deps serialize); the Shared-space AllGather outputs must be
        # single-writer, so those are allocated per rep in emit_body.
        st1_in = dram.tile([64, 4], F32)
        ag2_in = dram.tile([AG2_PER], F32)
        c2_in = dram.tile([2, B, C, SLAB], BF16)
        c2_ri = dram.tile([NCORES, 2, B, C, SLAB], BF16)
        c2_ro = dram.tile([2, B, C, SLAB], BF16)
        s2_in = dram.tile([2, B, C, SLAB], BF16)
        s2_ri = dram.tile([NCORES, 2, B, C, SLAB], BF16)
        s2_ro = dram.tile([2, B, C, SLAB], BF16)
        st2_in = dram.tile([64, 4], F32)
        fc_in = dram.tile([2, B, 2 * C, SLAB], BF16)
        fc_ri = dram.tile([NCORES, 2, B, 2 * C, SLAB], BF16)
        fc_ro = dram.tile([2, B, 2 * C, SLAB], BF16)
        stf_in = dram.tile([64, 2], F32)
        bcast_dram = dram.tile([B, SHARD], F32)

        singles_cm = tc.tile_pool(name="singles", bufs=1)
        singles = singles_cm.__enter__()

        ident = singles.tile([64, 64], BF16)
        make_identity(nc, ident[:])
        ident_f32 = singles.tile([64, 64], F32)
        make_identity(nc, ident_f32[:])

        # constants to SBUF
        qw_sb = singles.tile([65, 64], BF16)
        kw_sb = singles.tile([65, 64], BF16)
        vw_sb = singles.tile([65, 66], BF16)
        bnp = singles.tile([64, 10], F32)
        gam_p = singles.tile([1, 1], F32)
        gam_c_col = singles.tile([64, 1], F32)
        ones_row = singles.tile([1, 64], F32)
        nc.sync.dma_start(out=qw_sb[:], in_=qw_d[:])
        nc.sync.dma_start(out=kw_sb[:], in_=kw_d[:])
        nc.sync.dma_start(out=vw_sb[:], in_=vw_d[:])
        nc.sync.dma_start(out=bnp[:], in_=bnp_d[:])
        nc.sync.dma_start(out=gam_p[:], in_=gam_d[0:1, 0:1])
        nc.sync.dma_start(
            out=gam_c_col[:],
            in_=bass.AP(tensor=gam_d, offset=1, ap=[[0, 64], [1, 1]]),
        )
        nc.vector.memset(ones_row[:], 1.0)
        eps_col = singles.tile([64, 1], F32)
        nc.vector.memset(eps_col[:], EPS)
        zrow = singles.tile([128, SLAB], BF16)
        nc.vector.memset(zrow[:], 0.0)

        # big persistent activations
        acts_cm = tc.tile_pool(name="acts", bufs=1)
        acts = acts_cm.__enter__()
        x_sb = acts.tile([128, LOCVIEW], BF16)
        nc.sync.dma_start(out=x_sb[:], in_=x_loc[:])

        s1_own = [acts.tile([65, SHARD], F32, name=f"s1own{b}") for b in range(B)]
        s1_own_bf = [acts.tile([65, SHARD], BF16, name=f"s1ownbf{b}") for b in range(B)]
        c1_own = [acts.tile([64, SHARD], F32, name=f"c1own{b}") for b in range(B)]
        c1_own_bf = [acts.tile([64, SHARD], BF16, name=f"c1ownbf{b}") for b in range(B)]
        for b in range(B):
            nc.vector.memset(s1_own[b][64:65, :], 1.0)
            nc.vector.memset(s1_own_bf[b][64:65, :], 1.0)

        s1_pam = [acts.tile([65, N], BF16, name=f"s1pam{b}") for b in range(B)]
        for b in range(B):
            nc.vector.memset(s1_pam[b][64:65, :], 1.0)

        k_stack = acts.tile([128, N], BF16)
        q_stack = acts.tile([128, SHARD], BF16)
        vt_sb = [acts.tile([128, 32 * 66], BF16, name=f"vt{b}") for b in range(B)]

        # hoisted per-rep reusable activations (same name => same slot)
        c2both = acts.tile([128, SHARD], BF16)
        s2both = acts.tile([128, SHARD], BF16)
        c2_loc = acts.tile([128, LOCVIEW], BF16)
        s2_loc = acts.tile([128, LOCVIEW], BF16)
        fcat_own = acts.tile([128, B * SHARD], BF16)
        fcat_loc = [acts.tile([128, LOCVIEW], BF16, name=f"fl{b}") for b in range(B)]
        out_own = acts.tile([128, SHARD], F32)
        # zero the padded views once; reps only rewrite interiors/halos
        nc.gpsimd.memset(c2_loc[:], 0.0)
        nc.gpsimd.memset(s2_loc[:], 0.0)
        for b in range(B):
            nc.gpsimd.memset(fcat_loc[b][:], 0.0)

        wpool_cm = tc.tile_pool(name="wpool", bufs=2)
        wpool = wpool_cm.__enter__()

        stats_pool_cm = tc.tile_pool(name="stats", bufs=1)
        stats_pool = stats_pool_cm.__enter__()

        tmp_pool_cm = tc.tile_pool(name="tmp", bufs=2)
        tmp_pool = tmp_pool_cm.__enter__()

        # ---------------- helpers ----------------
        def load_wconv(dram_t, name):
            w = wpool.tile([128, 27, 64], BF16, tag="wconv", name=name)
            nc.sync.dma_start(out=w[:], in_=dram_t[:])
            return w

        def conv64(w_sb_t, act, psum_pool, tname):
            """3x3x3 conv over 64-ch padded local view for own 2 slabs.

            Returns per-batch compact raw-output tiles t[b] (64, 512) plus
            (sum, sumsq) stat columns (64,1) each."""
            touts = []
            stats = []
            for b in range(B):
                t = stats_pool.tile([64, SHARD], F32, name=f"{tname}_t{b}")
                for jj, jslab in enumerate((1, 2)):
                    ps = psum_pool.tile(
                        [64, ROW], F32, tag=f"convps{b}", name=f"{tname}ps{b}{jj}"
                    )
                    base = LOCPAD + jslab * ROW
                    for o in range(27):
                        nc.tensor.matmul(
                            ps[:],
                            lhsT=w_sb_t[64 * b : 64 * b + 64, o, :],
                            rhs=act[
                                64 * b : 64 * b + 64,
                                base + DELTAS[o] : base + DELTAS[o] + ROW,
                            ],
                            start=(o == 0),
                            stop=(o == 26),
                            tile_position=(64 * b, 0),
                        )
                    nc.vector.tensor_copy(
                        t[:, jj * SLAB : (jj + 1) * SLAB],
                        ps[:, :].rearrange("p (w d) -> p w d", w=18)[
                            :, 1:17, 1:17
                        ],
                    )
                touts.append(t)
                ssum = stats_pool.tile([64, 1], F32, name=f"{tname}_s{b}")
                ssq = stats_pool.tile([64, 1], F32, name=f"{tname}_q{b}")
                scr2 = tmp_pool.tile([64, SHARD], F32, tag="scrB", name=f"{tname}scrB{b}")
                nc.vector.reduce_sum(ssum[:], t[:], axis=AX.X)
                nc.scalar.activation(scr2[:], t[:], AF.Square, accum_out=ssq[:])
                stats.append((ssum, ssq))
            return touts, stats

        def conv128(w_sb_t, act_pair, psum_pool, tname):
            """3x3x3 conv with 128 input channels (fused concat), per batch."""
            touts = []
            stats = []
            for b in range(B):
                t = stats_pool.tile([64, SHARD], F32, name=f"{tname}_t{b}")
                for jj, jslab in enumerate((1, 2)):
                    ps = psum_pool.tile(
                        [64, ROW], F32, tag=f"convps{b}", name=f"{tname}ps{b}{jj}"
                    )
                    base = LOCPAD + jslab * ROW
                    for o in range(27):
                        nc.tensor.matmul(
                            ps[:],
                            lhsT=w_sb_t[:, o, :],
                            rhs=act_pair[b][
                                :, base + DELTAS[o] : base + DELTAS[o] + ROW
                            ],
                            start=(o == 0),
                            stop=(o == 26),
                        )
                    nc.vector.tensor_copy(
                        t[:, jj * SLAB : (jj + 1) * SLAB],
                        ps[:, :].rearrange("p (w d) -> p w d", w=18)[
                            :, 1:17, 1:17
                        ],
                    )
                touts.append(t)
                ssum = stats_pool.tile([64, 1], F32, name=f"{tname}_s{b}")
                ssq = stats_pool.tile([64, 1], F32, name=f"{tname}_q{b}")
                scr2 = tmp_pool.tile([64, SHARD], F32, tag="scrB", name=f"{tname}scrB{b}")
                nc.vector.reduce_sum(ssum[:], t[:], axis=AX.X)
                nc.scalar.activation(scr2[:], t[:], AF.Square, accum_out=ssq[:])
                stats.append((ssum, ssq))
            return touts, stats

        def pack_stats(dst_sb, stats_list):
            """stats_list: list of (ssum_b0, ssq_b0), (ssum_b1, ssq_b1) pairs
            per conv; writes [sum, sq] per conv into dst columns."""
            for ci, st in enumerate(stats_list):
                (s0, q0), (s1_, q1) = st
                nc.vector.tensor_add(dst_sb[:, 2 * ci : 2 * ci + 1], s0[:], s1_[:])
                nc.vector.tensor_add(
                    dst_sb[:, 2 * ci + 1 : 2 * ci + 2], q0[:], q1[:]
                )

        def bn_coeffs(tot_sb, col, g_col, b_col, name):
            """From total [sum, sumsq] cols compute A=(g*rstd), B=b-mean*A and
            the rrelu-scaled variants. Returns (A, B, As, Bs) (64,1) tiles."""
            mean = stats_pool.tile([64, 1], F32, name=f"{name}_mean")
            var = stats_pool.tile([64, 1], F32, name=f"{name}_var")
            a_t = stats_pool.tile([64, 1], F32, name=f"{name}_A")
            b_t = stats_pool.tile([64, 1], F32, name=f"{name}_B")
            as_t = stats_pool.tile([64, 1], F32, name=f"{name}_As")
            bs_t = stats_pool.tile([64, 1], F32, name=f"{name}_Bs")
            scr = stats_pool.tile([64, 1], F32, name=f"{name}_scr")
            nc.vector.tensor_scalar(
                mean[:], tot_sb[:, col : col + 1], 1.0 / NTOT, None, ALU.mult
            )
            nc.vector.tensor_scalar(
                var[:], tot_sb[:, col + 1 : col + 2], 1.0 / NTOT, None, ALU.mult
            )
            nc.vector.tensor_mul(scr[:], mean[:], mean[:])
            nc.vector.tensor_sub(var[:], var[:], scr[:])
            # rstd = exp(-0.5*ln(var+eps)); avoids the Sqrt table set
            nc.scalar.activation(scr[:], var[:], AF.Ln, bias=eps_col[:])
            nc.vector.tensor_scalar(scr[:], scr[:], -0.5, None, ALU.mult)
            nc.scalar.activation(scr[:], scr[:], AF.Exp)
            nc.vector.tensor_mul(a_t[:], scr[:], g_col)
            nc.vector.tensor_mul(scr[:], mean[:], a_t[:])
            nc.vector.tensor_sub(b_t[:], b_col, scr[:])
            nc.vector.tensor_scalar(as_t[:], a_t[:], SLOPE, None, ALU.mult)
            nc.vector.tensor_scalar(bs_t[:], b_t[:], SLOPE, None, ALU.mult)
            return a_t, b_t, as_t, bs_t

        def bn_rrelu(t_raw, coeffs, dst_ap):
            """dst = max(A*t+B, As*t+Bs) elementwise."""
            a_t, b_t, as_t, bs_t = coeffs
            y1 = tmp_pool.tile([64, SHARD], F32, tag="y1", name="y1_t")
            y2 = tmp_pool.tile([64, SHARD], F32, tag="y2", name="y2_t")
            nc.vector.tensor_scalar(
                y1[:], t_raw[:], a_t[:], b_t[:], ALU.mult, ALU.add
            )
            nc.vector.tensor_scalar(
                y2[:], t_raw[:], as_t[:], bs_t[:], ALU.mult, ALU.add
            )
            nc.vector.tensor_max(dst_ap, y1[:], y2[:])

        def halo_exchange(in_t, ag_t, ri_t, ro_t, nch):
            """AG own slabs, then RS-rotate so each core receives exactly its
            lo/hi halo slabs (slot-static reads of the gathered buffer)."""
            nc.gpsimd.collective_compute(
                "AllGather", ALU.bypass, replica_groups=rg,
                ins=[in_t[:].opt()], outs=[ag_t[:].opt()],
            )
            blk = B * nch * SLAB  # one slab block (elements)
            per = 2 * blk  # one rank contribution
            for i in range(NCORES):
                # lo slot: rank i-1's slab 1
                if i > 0:
                    nc.sync.dma_start(
                        out=bass.AP(
                            tensor=ri_t[:].tensor,
                            offset=i * per,
                            ap=[[1, blk]],
                        ),
                        in_=bass.AP(
                            tensor=ag_t[:].tensor,
                            offset=(i - 1) * per + blk,
                            ap=[[1, blk]],
                        ),
                    )
                else:
                    for z in range(blk // (128 * SLAB)):
                        nc.sync.dma_start(
                            out=bass.AP(
                                tensor=ri_t[:].tensor,
                                offset=z * 128 * SLAB,
                                ap=[[SLAB, 128], [1, SLAB]],
                            ),
                            in_=zrow[:],
                        )
                # hi slot: rank i+1's slab 0
                if i < NCORES - 1:
                    nc.sync.dma_start(
                        out=bass.AP(
                            tensor=ri_t[:].tensor,
                            offset=i * per + blk,
                            ap=[[1, blk]],
                        ),
                        in_=bass.AP(
                            tensor=ag_t[:].tensor,
                            offset=(i + 1) * per,
                            ap=[[1, blk]],
                        ),
                    )
                else:
                    for z in range(blk // (128 * SLAB)):
                        nc.sync.dma_start(
                            out=bass.AP(
                                tensor=ri_t[:].tensor,
                                offset=i * per + blk + z * 128 * SLAB,
                                ap=[[SLAB, 128], [1, SLAB]],
                            ),
                            in_=zrow[:],
                        )
            nc.gpsimd.collective_compute(
                "ReduceScatter", ALU.add, replica_groups=rg,
                ins=[ri_t[:].opt()], outs=[ro_t[:].opt()],
            )

        def build_view(ro_t, nch, bsel, dst, own_ap, name):
            """dst (128, LOCVIEW) bf16: slabs 1-2 <- own; 0/3 <- RS halos/8."""
            blk = B * nch * SLAB
            boff = 0 if bsel is None else bsel * nch * SLAB
            for dslab, hs in ((0, 0), (3, 1)):
                stg = tmp_pool.tile(
                    [128, SLAB], BF16, tag="hstg", name=f"hs{name}{dslab}"
                )
                nc.sync.dma_start(
                    out=stg[:],
                    in_=bass.AP(
                        tensor=ro_t[:].tensor,
                        offset=hs * blk + boff,
                        ap=[[SLAB, 128], [1, SLAB]],
                    ),
                )
                nc.vector.tensor_scalar(
                    dst[:, LOCPAD + dslab * ROW : LOCPAD + (dslab + 1) * ROW]
                    .rearrange("p (w d) -> p w d", w=18)[:, 1:17, 1:17],
                    stg[:].rearrange("p (w d) -> p w d", w=16),
                    1.0 / NCORES,
                    None,
                    ALU.mult,
                )
            nc.vector.tensor_copy(
                dst[:, LOCPAD + 1 * ROW : LOCPAD + 3 * ROW]
                .rearrange("p (j w d) -> p j w d", j=2, w=18)[:, :, 1:17, 1:17],
                own_ap,
            )

        def emit_body(rep):
            # per-rep Shared collective outputs (single-writer rule)
            st1_out = dram.tile([NCORES, 64, 4], F32, addr_space="Shared",
                                name=f"st1_out_r{rep}")
            ag2_out = dram.tile([NCORES * AG2_PER], F32, addr_space="Shared",
                                name=f"ag2_out_r{rep}")
            c2_ag = dram.tile([NCORES, 2, B, C, SLAB], BF16,
                              addr_space="Shared", name=f"c2_ag_r{rep}")
            s2_ag = dram.tile([NCORES, 2, B, C, SLAB], BF16,
                              addr_space="Shared", name=f"s2_ag_r{rep}")
            st2_out = dram.tile([NCORES, 64, 4], F32, addr_space="Shared",
                                name=f"st2_out_r{rep}")
            fc_ag = dram.tile([NCORES, 2, B, 2 * C, SLAB], BF16,
                              addr_space="Shared", name=f"fc_ag_r{rep}")
            stf_out = dram.tile([NCORES, 64, 2], F32, addr_space="Shared",
                                name=f"stf_out_r{rep}")

            # =========== phase 1: conv S and conv C (input x) ===========
            cpsum_cm = tc.tile_pool(name=f"cpsum_r{rep}", bufs=2, space="PSUM")
            cpsum = cpsum_cm.__enter__()

            ws_sb = load_wconv(w_s, "wsS")
            tS, statS = conv64(ws_sb, x_sb, cpsum, "cS")
            wc_sb = load_wconv(w_c, "wsC")
            tC, statC = conv64(wc_sb, x_sb, cpsum, "cC")

            st1_sb = stats_pool.tile([64, 4], F32, name="st1_sb")
            pack_stats(st1_sb, [statS, statC])
            nc.sync.dma_start(out=st1_in[:], in_=st1_sb[:])
            nc.gpsimd.collective_compute(
                "AllGather",
                ALU.bypass,
                replica_groups=rg,
                ins=[st1_in[:].opt()],
                outs=[st1_out[:].opt()],
            )

            # reduce gathered stats and compute BN coefficients
            st1_stage = stats_pool.tile([64, 4, NCORES], F32, name="st1_stage")
            nc.sync.dma_start(
                out=st1_stage[:],
                in_=bass.AP(
                    tensor=st1_out[:].tensor,
                    offset=0,
                    ap=[[4, 64], [1, 4], [256, NCORES]],
                ),
            )
            st1_tot = stats_pool.tile([64, 4], F32, name="st1_tot")
            nc.vector.tensor_reduce(st1_tot[:], st1_stage[:], axis=AX.X, op=ALU.add)
            cS = bn_coeffs(st1_tot, 0, bnp[:, 0:1], bnp[:, 1:2], "bnS")
            cC = bn_coeffs(st1_tot, 2, bnp[:, 2:3], bnp[:, 3:4], "bnC")

            for b in range(B):
                bn_rrelu(tS[b], cS, s1_own[b][0:64, :])
                bn_rrelu(tC[b], cC, c1_own[b][:, :])
                nc.vector.tensor_copy(s1_own_bf[b][0:64, :], s1_own[b][0:64, :])
                nc.vector.tensor_copy(c1_own_bf[b][:, :], c1_own[b][:, :])

            cpsum_cm.__exit__(None, None, None)

            # =========== phase 2: CAM partial gram + AG2 (s1 + gram) ===========
            mpsum_cm = tc.tile_pool(name=f"mpsum_r{rep}", bufs=2, space="PSUM")
            mpsum = mpsum_cm.__enter__()

            ft_sb = [tmp_pool.tile([128, 4 * 64], BF16, tag=f"ft{b}", name=f"ft{b}") for b in range(B)]
            gram_sb = tmp_pool.tile([64, B * 64], F32, tag="gram", name="gram_sb")
            for b in range(B):
                for kk in range(4):
                    pst = mpsum.tile([128, 64], BF16, tag="mm", name=f"ft{b}{kk}")
                    nc.tensor.transpose(
                        pst[:],
                        c1_own_bf[b][:, 128 * kk : 128 * (kk + 1)],
                        ident[:],
                    )
                    nc.vector.tensor_copy(
                        ft_sb[b][:, 64 * kk : 64 * (kk + 1)], pst[:, 0:64]
                    )
                psg = mpsum.tile([64, 64], F32, tag="mm", name=f"gram{b}")
                for kk in range(4):
                    nc.tensor.matmul(
                        psg[:],
                        lhsT=ft_sb[b][:, 64 * kk : 64 * (kk + 1)],
                        rhs=ft_sb[b][:, 64 * kk : 64 * (kk + 1)],
                        start=(kk == 0),
                        stop=(kk == 3),
                    )
                nc.vector.tensor_copy(gram_sb[:, 64 * b : 64 * (b + 1)], psg[:])

            # write AG2 contribution: s1 (slab-major) + gram
            for b in range(B):
                nc.sync.dma_start(
                    out=bass.AP(
                        tensor=ag2_in[:].tensor,
                        offset=b * C * SLAB,
                        ap=[[SLAB, 64], [B * C * SLAB, 2], [1, SLAB]],
                    ),
                    in_=s1_own[b][0:64, :].rearrange("p (j s) -> p j s", j=2),
                )
            nc.sync.dma_start(
                out=bass.AP(
                    tensor=ag2_in[:].tensor,
                    offset=AG2_S1,
                    ap=[[64, 64], [64 * 64, B], [1, 64]],
                ),
                in_=gram_sb[:].rearrange("p (b c) -> p b c", b=B),
            )
            nc.gpsimd.collective_compute(
                "AllGather",
                ALU.bypass,
                replica_groups=rg,
                ins=[ag2_in[:].opt()],
                outs=[ag2_out[:].opt()],
            )

            # =========== phase 3: q (local), then k/vT from gathered s1 ===========
            for b in range(B):
                psq = mpsum.tile([64, SHARD], F32, tag="qk", name=f"q{b}")
                nc.tensor.matmul(
                    psq[:],
                    lhsT=qw_sb[:],
                    rhs=s1_own_bf[b][:],
                    start=True,
                    stop=True,
                )
                nc.vector.tensor_copy(q_stack[64 * b : 64 * (b + 1), :], psq[:])

            # load gathered s1 into s1_pam (global n order); one DMA per slab half
            for b in range(B):
                for j in range(2):
                    nc.gpsimd.dma_start(
                        out=s1_pam[b][0:64, :]
                        .rearrange("p (g s) -> p g s", s=2 * SLAB)[:, :, j * SLAB : (j + 1) * SLAB],
                        in_=bass.AP(
                            tensor=ag2_out[:].tensor,
                            offset=b * C * SLAB + j * B * C * SLAB,
                            ap=[[SLAB, 64], [AG2_PER, NCORES], [1, SLAB]],
                        ),
                    )
            # gathered gram -> reduce over cores
            gram_full = [tmp_pool.tile([64, 64], F32, tag=f"gramf{b}", name=f"gramf{b}") for b in range(B)]
            for b in range(B):
                gstage = tmp_pool.tile(
                    [64, 64, NCORES], F32, tag="gstage", name=f"gstage{b}"
                )
                nc.sync.dma_start(
                    out=gstage[:],
                    in_=bass.AP(
                        tensor=ag2_out[:].tensor,
                        offset=AG2_S1 + b * C * C,
                        ap=[[64, 64], [1, 64], [AG2_PER, NCORES]],
                    ),
                )
                nc.vector.tensor_reduce(gram_full[b][:], gstage[:], axis=AX.X, op=ALU.add)

            for b in range(B):
                for nt in range(8):
                    psk = mpsum.tile([64, 512], F32, tag="qk", name=f"k{b}{nt}")
                    nc.tensor.matmul(
                        psk[:],
                        lhsT=kw_sb[:],
                        rhs=s1_pam[b][:, 512 * nt : 512 * (nt + 1)],
                        start=True,
                        stop=True,
                    )
                    nc.vector.tensor_copy(
                        k_stack[64 * b : 64 * (b + 1), 512 * nt : 512 * (nt + 1)],
                        psk[:],
                    )
                for mt in range(32):
                    psv = mpsum.tile([128, 66], F32, tag="vt", name=f"v{b}{mt}")
                    nc.tensor.matmul(
                        psv[:],
                        lhsT=s1_pam[b][:, 128 * mt : 128 * (mt + 1)],
                        rhs=vw_sb[:],
                        start=True,
                        stop=True,
                    )
                    nc.vector.tensor_copy(
                        vt_sb[b][:, 66 * mt : 66 * (mt + 1)], psv[:]
                    )

            # =========== phase 4: CAM finish -> c2 -> pair halo AG ===========
            for b in range(B):
                rowmax = tmp_pool.tile([64, 1], F32, tag="camx", name=f"camx{b}")
                den = tmp_pool.tile([64, 1], F32, tag="camd", name=f"camd{b}")
                attn = tmp_pool.tile([64, 64], F32, tag="cama", name=f"cama{b}")
                nc.vector.tensor_reduce(
                    rowmax[:], gram_full[b][:], axis=AX.X, op=ALU.min
                )
                nc.scalar.activation(
                    attn[:],
                    gram_full[b][:],
                    AF.Exp,
                    bias=rowmax[:],
                    scale=-1.0,
                    accum_out=den[:],
                )
                nc.vector.reciprocal(den[:], den[:])
                nc.vector.tensor_scalar(attn[:], attn[:], den[:], None, ALU.mult)
                # attn^T via PE
                psat = mpsum.tile([64, 64], F32, tag="mm", name=f"at{b}")
                nc.tensor.transpose(psat[:], attn[:], ident_f32[:])
                attnT = tmp_pool.tile([64, 64], BF16, tag="camat", name=f"camat{b}")
                nc.vector.tensor_copy(attnT[:], psat[:])
                # cam_out = attnT.T @ c1_own
                psco = mpsum.tile([64, SHARD], F32, tag="qk", name=f"co{b}")
                nc.tensor.matmul(
                    psco[:],
                    lhsT=attnT[:],
                    rhs=c1_own_bf[b][:],
                    start=True,
                    stop=True,
                )
                c2t = tmp_pool.tile([64, SHARD], F32, tag="c2t", name=f"c2t{b}")
                nc.vector.tensor_scalar(c2t[:], psco[:], gam_c_col[:, 0:1], None, ALU.mult)
                nc.vector.tensor_add(
                    c2both[64 * b : 64 * (b + 1), :], c2t[:], c1_own[b][:]
                )
                nc.sync.dma_start(
                    out=bass.AP(
                        tensor=c2_in[:].tensor,
                        offset=b * C * SLAB,
                        ap=[[SLAB, 64], [B * C * SLAB, 2], [1, SLAB]],
                    ),
                    in_=c2both[64 * b : 64 * (b + 1), :].rearrange(
                        "p (j s) -> p j s", j=2
                    ),
                )
            halo_exchange(c2_in, c2_ag, c2_ri, c2_ro, C)

            mpsum_cm.__exit__(None, None, None)

            # =========== phase 5: PAM attention ===========
            epsum_cm = tc.tile_pool(name=f"epsum_r{rep}", bufs=3, space="PSUM")
            epsum = epsum_cm.__enter__()
            opsum_cm = tc.tile_pool(name=f"opsum_r{rep}", bufs=1, space="PSUM")
            opsum = opsum_cm.__enter__()
            apool_cm = tc.tile_pool(name=f"apool_r{rep}", bufs=3)
            apool = apool_cm.__enter__()

            o_ps = [
                opsum.tile([65, SHARD], F32, name=f"ops{b}", tag=f"ops{b}")
                for b in range(B)
            ]
            for g2 in range(16):
                for b in range(B):
                    e_ps = epsum.tile([128, 1024], F32, tag="eg", name=f"e{g2}{b}")
                    for j in range(2):
                        mt = 2 * g2 + j
                        nc.tensor.matmul(
                            e_ps[:, 512 * j : 512 * (j + 1)],
                            lhsT=k_stack[
                                64 * b : 64 * (b + 1), 128 * mt : 128 * (mt + 1)
                            ],
                            rhs=q_stack[64 * b : 64 * (b + 1), :],
                            start=True,
                            stop=True,
                            tile_position=(64 * b, 0),
                        )
                    a_sb = apool.tile([128, 1024], BF16, tag="ag", name=f"a{g2}{b}")
                    nc.scalar.activation(a_sb[:], e_ps[:], AF.Exp)
                    for j in range(2):
                        mt = 2 * g2 + j
                        nc.tensor.matmul(
                            o_ps[b][:],
                            lhsT=vt_sb[b][:, 66 * mt : 66 * mt + 65],
                            rhs=a_sb[:, 512 * j : 512 * (j + 1)],
                            start=(mt == 0),
                            stop=(mt == 31),
                        )

            # =========== phase 6: PAM finalize -> s2 -> pair halo AG ===========
            for b in range(B):
                recip = tmp_pool.tile([1, SHARD], F32, tag="rec", name=f"rec{b}")
                recipg = tmp_pool.tile([1, SHARD], F32, tag="recg", name=f"recg{b}")
                nc.vector.reciprocal(recip[:], o_ps[b][64:65, :])
                nc.vector.tensor_scalar(
                    recipg[:], recip[:], gam_p[:, 0:1], None, ALU.mult
                )
                nc.sync.dma_start(out=bcast_dram[b : b + 1, :], in_=recipg[:])
                bc_sb = tmp_pool.tile([64, SHARD], F32, tag="bcs", name=f"bcs{b}")
                nc.sync.dma_start(
                    out=bc_sb[:],
                    in_=bass.AP(
                        tensor=bcast_dram[:].tensor,
                        offset=b * SHARD,
                        ap=[[0, 64], [1, SHARD]],
                    ),
                )
                s2t = tmp_pool.tile([64, SHARD], F32, tag="s2t", name=f"s2t{b}")
                nc.vector.tensor_mul(s2t[:], o_ps[b][0:64, :], bc_sb[:])
                nc.vector.tensor_add(
                    s2both[64 * b : 64 * (b + 1), :], s2t[:], s1_own[b][0:64, :]
                )
                nc.sync.dma_start(
                    out=bass.AP(
                        tensor=s2_in[:].tensor,
                        offset=b * C * SLAB,
                        ap=[[SLAB, 64], [B * C * SLAB, 2], [1, SLAB]],
                    ),
                    in_=s2both[64 * b : 64 * (b + 1), :].rearrange(
                        "p (j s) -> p j s", j=2
                    ),
                )
            halo_exchange(s2_in, s2_ag, s2_ri, s2_ro, C)

            for p in (apool_cm, opsum_cm, epsum_cm):
                p.__exit__(None, None, None)
            cpsum_cm = tc.tile_pool(name=f"cpsum2_r{rep}", bufs=2, space="PSUM")
            cpsum = cpsum_cm.__enter__()

            # =========== phase 7: conv C1 (on gathered c2) ===========
            build_view(
                c2_ro, C, None, c2_loc,
                c2both[:].rearrange("p (j w d) -> p j w d", j=2, w=16), "c2",
            )
            wc1_sb = load_wconv(w_c1, "wsC1")
            tC1, statC1 = conv64(wc1_sb, c2_loc, cpsum, "cC1")

            # =========== phase 8: conv S1 (on gathered s2) ===========
            build_view(
                s2_ro, C, None, s2_loc,
                s2both[:].rearrange("p (j w d) -> p j w d", j=2, w=16), "s2",
            )
            ws1_sb = load_wconv(w_s1, "wsS1")
            tS1, statS1 = conv64(ws1_sb, s2_loc, cpsum, "cS1")

            st2_sb = stats_pool.tile([64, 4], F32, name="st2_sb")
            pack_stats(st2_sb, [statS1, statC1])
            nc.sync.dma_start(out=st2_in[:], in_=st2_sb[:])
            nc.gpsimd.collective_compute(
                "AllGather",
                ALU.bypass,
                replica_groups=rg,
                ins=[st2_in[:].opt()],
                outs=[st2_out[:].opt()],
            )
            st2_stage = stats_pool.tile([64, 4, NCORES], F32, name="st2_stage")
            nc.sync.dma_start(
                out=st2_stage[:],
                in_=bass.AP(
                    tensor=st2_out[:].tensor,
                    offset=0,
                    ap=[[4, 64], [1, 4], [256, NCORES]],
                ),
            )
            st2_tot = stats_pool.tile([64, 4], F32, name="st2_tot")
            nc.vector.tensor_reduce(st2_tot[:], st2_stage[:], axis=AX.X, op=ALU.add)
            cS1 = bn_coeffs(st2_tot, 0, bnp[:, 4:5], bnp[:, 5:6], "bnS1")
            cC1 = bn_coeffs(st2_tot, 2, bnp[:, 6:7], bnp[:, 7:8], "bnC1")

            for b in range(B):
                bn_rrelu(tS1[b], cS1, fcat_own[0:64, b * SHARD : (b + 1) * SHARD])
                bn_rrelu(tC1[b], cC1, fcat_own[64:128, b * SHARD : (b + 1) * SHARD])

            # contribution: (2 slab, 2 b, 128 c, 256); one DMA per batch
            for b in range(B):
                nc.sync.dma_start(
                    out=bass.AP(
                        tensor=fc_in[:].tensor,
                        offset=b * 2 * C * SLAB,
                        ap=[[SLAB, 128], [B * 2 * C * SLAB, 2], [1, SLAB]],
                    ),
                    in_=fcat_own[:, b * SHARD : (b + 1) * SHARD].rearrange(
                        "p (j s) -> p j s", j=2
                    ),
                )
            halo_exchange(fc_in, fc_ag, fc_ri, fc_ro, 2 * C)

            # =========== phase 9: conv F ===========
            for b in range(B):
                build_view(
                    fc_ro, 2 * C, b, fcat_loc[b],
                    fcat_own[:, b * SHARD : (b + 1) * SHARD].rearrange(
                        "p (j w d) -> p j w d", j=2, w=16
                    ),
                    f"fc{b}",
                )
            wf_sb = load_wconv(w_f, "wsF")
            tF, statF = conv128(wf_sb, fcat_loc, cpsum, "cF")

            stf_sb = stats_pool.tile([64, 2], F32, name="stf_sb")
            pack_stats(stf_sb, [statF])
            nc.sync.dma_start(out=stf_in[:], in_=stf_sb[:])
            nc.gpsimd.collective_compute(
                "AllGather",
                ALU.bypass,
                replica_groups=rg,
                ins=[stf_in[:].opt()],
                outs=[stf_out[:].opt()],
            )
            stf_stage = stats_pool.tile([64, 2, NCORES], F32, name="stf_stage")
            nc.sync.dma_start(
                out=stf_stage[:],
                in_=bass.AP(
                    tensor=stf_out[:].tensor,
                    offset=0,
                    ap=[[2, 64], [1, 2], [128, NCORES]],
                ),
            )
            stf_tot = stats_pool.tile([64, 2], F32, name="stf_tot")
            nc.vector.tensor_reduce(stf_tot[:], stf_stage[:], axis=AX.X, op=ALU.add)
            cF = bn_coeffs(stf_tot, 0, bnp[:, 8:9], bnp[:, 9:10], "bnF")

            for b in range(B):
                bn_rrelu(tF[b], cF, out_own[64 * b : 64 * (b + 1), :])
            nc.sync.dma_start(
                out=bass.AP(
                    tensor=out_d,
                    offset=0,
                    ap=[[SHARD, 128], [1, SHARD]],
                ),
                in_=out_own[:],
            )
            cpsum_cm.__exit__(None, None, None)

        for rep in range(reps):
            emit_body(rep)

        for p in (tmp_pool_cm, stats_pool_cm, wpool_cm, acts_cm,
                  singles_cm, dram_cm):
            p.__exit__(None, None, None)

    nc.finalize()
    return nc


def _prep_host(inputs):
    """Build per-core in_maps from the full problem inputs."""
    x = np.asarray(inputs["x"], np.float32)

    import ml_dtypes

    def conv_wT(w):
        # w: (O, I, 3, 3, 3) -> (128, 27, 64): [dup*64+i, off, o]
        wt = np.transpose(np.asarray(w, np.float32), (1, 2, 3, 4, 0)).reshape(
            w.shape[1], 27, 64
        )
        if w.shape[1] == 64:
            wt = np.concatenate([wt, wt], axis=0)
        return wt.astype(ml_dtypes.bfloat16)

    qw = np.asarray(inputs["qw"], np.float32).reshape(64, 64)
    kw = np.asarray(inputs["kw"], np.float32).reshape(64, 64)
    vw = np.asarray(inputs["vw"], np.float32).reshape(64, 64)
    qa = np.zeros((65, 64), np.float32)
    qa[:64] = qw.T
    qa[64] = np.asarray(inputs["qb"], np.float32)
    ka = np.zeros((65, 64), np.float32)
    ka[:64] = kw.T
    ka[64] = np.asarray(inputs["kb"], np.float32)
    va = np.zeros((65, 66), np.float32)
    va[:64, :64] = vw.T
    va[64, :64] = np.asarray(inputs["vb"], np.float32)
    va[64, 64] = 1.0

    bnp = np.stack(
        [
            np.asarray(inputs[k], np.float32)
            for k in ("gS", "bS", "gC", "bC", "gS1", "bS1", "gC1", "bC1", "gF", "bF")
        ],
        axis=1,
    )
    gam = np.array(
        [[float(np.asarray(inputs["gamma_p"]).reshape(-1)[0]),
          float(np.asarray(inputs["gamma_c"]).reshape(-1)[0])]],
        np.float32,
    )

    shared = {
        "w_s": conv_wT(inputs["wS"]),
        "w_c": conv_wT(inputs["wC"]),
        "w_s1": conv_wT(inputs["wS1"]),
        "w_c1": conv_wT(inputs["wC1"]),
        "w_f": conv_wT(inputs["wF"]),
        "qw": qa.astype(ml_dtypes.bfloat16),
        "kw": ka.astype(ml_dtypes.bfloat16),
        "vw": va.astype(ml_dtypes.bfloat16),
        "bnp": np.ascontiguousarray(bnp),
        "gam": gam,
    }

    # padded full volume (h, w, d) -> (18, 18, 18), flattened per (b, c)
    xp = np.zeros((B, C, 18, 18, 18), np.float32)
    xp[:, :, 1:17, 1:17, 1:17] = x
    xp = xp.reshape(B, C, 18, ROW)

    in_maps = []
    for i in range(NCORES):
        xl = np.zeros((128, LOCVIEW), np.float32)
        for j in range(4):
            g = 2 * i - 1 + j  # global h-slab (padded index g+1)
            view = xp[:, :, g + 1].reshape(128, ROW)
            xl[:, LOCPAD + j * ROW : LOCPAD + (j + 1) * ROW] = view
        m = dict(shared)
        m["x_loc"] = xl.astype(ml_dtypes.bfloat16)
        in_maps.append(m)
    return in_maps


class _Exec:
    """Compile-once executor: bass program + cached jitted PJRT callable."""

    def __init__(self, reps):
        import jax
        from jax.sharding import Mesh, PartitionSpec, NamedSharding
        from jax.experimental.shard_map import shard_map
        from concourse.bass2jax import (
            _bass_exec_p,
            install_neuronx_cc_hook,
            partition_id_tensor,
        )

        self.jax = jax
        self.nc = build_program(reps)
        install_neuronx_cc_hook()
        nc_ = self.nc
        partition_name = (
            nc_.partition_id_tensor.name if nc_.partition_id_tensor else None
        )
        in_names, out_names, out_avals, zero_shapes = [], [], [], []
        for alloc in nc_.m.functions[0].allocations:
            if not isinstance(alloc, mybir.MemoryLocationSet):
                continue
            name = alloc.memorylocations[0].name
            if alloc.kind == "ExternalInput":
                if name != partition_name:
                    in_names.append(name)
            elif alloc.kind == "ExternalOutput":
                shape = tuple(alloc.tensor_shape)
                dtype = mybir.dt.np(alloc.dtype)
                out_names.append(name)
                out_avals.append(jax.core.ShapedArray(shape, dtype))
                zero_shapes.append((shape, dtype))
        self.in_names = in_names
        self.out_names = out_names
        self.out_avals = out_avals
        self.zero_shapes = zero_shapes
        n_params = len(in_names)
        n_outs = len(out_avals)
        self.n_params = n_params
        all_in = in_names + out_names + ([partition_name] if partition_name else [])

        def _body(*args):
            operands = list(args)
            if partition_name:
                operands.append(partition_id_tensor())
            return tuple(
                _bass_exec_p.bind(
                    *operands,
                    out_avals=tuple(out_avals),
                    in_names=tuple(all_in),
                    out_names=tuple(out_names),
                    lowering_input_output_aliases=(),
                    sim_require_finite=True,
                    sim_require_nnan=True,
                    nc=nc_,
                )
            )

        devices = jax.devices()[:NCORES]
        assert len(devices) == NCORES
        self.mesh = Mesh(np.asarray(devices), ("core",))
        self.psharded = NamedSharding(self.mesh, PartitionSpec("core"))
        self.sharded = jax.jit(
            shard_map(
                _body,
                mesh=self.mesh,
                in_specs=(PartitionSpec("core"),) * (n_params + n_outs),
                out_specs=(PartitionSpec("core"),) * n_outs,
                check_rep=False,
            ),
            donate_argnums=tuple(range(n_params, n_params + n_outs)),
            keep_unused=True,
        )

    def concat_inputs(self, in_maps):
        return [
            np.concatenate([np.asarray(m[name]) for m in in_maps], axis=0)
            for name in self.in_names
        ]

    def zeros(self):
        return [
            np.zeros((NCORES * s[0], *s[1:]), d) for (s, d) in self.zero_shapes
        ]

    def run(self, in_maps):
        """Full path: numpy in -> per-core dict of numpy outputs."""
        concat_in = self.concat_inputs(in_maps)
        out_arrs = self.sharded(*concat_in, *self.zeros())
        return [
            {
                name: np.asarray(out_arrs[i]).reshape(
                    NCORES, *self.out_avals[i].shape
                )[c]
                for i, name in enumerate(self.out_names)
            }
            for c in range(NCORES)
        ]


_EXECS = {}


def _get_exec(reps=1):
    if reps not in _EXECS:
        _EXECS[reps] = _Exec(reps)
    return _EXECS[reps]


def _run_fallback(inputs):
    """Stock path for non-axon environments."""
    from concourse.bass_utils import run_bass_kernel_spmd

    if "nc" not in _EXECS:
        _EXECS["nc"] = build_program(1)
    nc_ = _EXECS["nc"]
    in_maps = _prep_host(inputs)
    return run_bass_kernel_spmd(nc_, in_maps, list(range(NCORES))).results


def kernel(**inputs) -> np.ndarray:
    from concourse._compat import axon_active

    if axon_active():
        ex = _get_exec(1)
        res = ex.run(_prep_host(inputs))
    else:
        res = _run_fallback(inputs)
    out = np.zeros((B, C, HH, HH, HH), np.float32)
    ov = out.reshape(B, C, 8, 2, SLAB)
    for i in range(NCORES):
        ov[:, :, i] = res[i]["out"].reshape(B, C, 2, SLAB)
    return out


if __name__ == "__main__":
    rng = np.random.default_rng(0)
    print("building program...")
    nc = build_program()
    print("ok")


# revision 8
# speedup vs baseline: 3982.1994x; 1.5365x over previous
"""Trainium2 Bass kernel for DAResBlock3D (dual-attention residual block).

Strategy (8 NeuronCores, SPMD):
  - Spatial sharding over H: core i owns output h-slabs {2i, 2i+1} (512 of
    4096 positions per batch), both batches on-chip as partition halves.
  - 3x3x3 convs: 27 shifted matmuls over a zero-padded local view (4 h-slabs
    with halo).  Phase-1 convS/convC are column-packed into one matmul
    stream (shared rhs = x, stacked S|C output channels); batches run
    concurrently in separate PE row bands (tile_position).
  - Halo exchange: AllGather of own slabs, then each core reads exactly its
    neighbours' slabs from the gathered buffer with dynamic
    (partition_id-indexed) conditional DMAs.  Halo-dependent conv offsets
    are ordered last in each PSUM accumulation chain so the conv overlaps
    the in-flight AllGather.
  - BatchNorm (train-mode, global stats): per-core partial sums AllGathered
    and reduced redundantly on every core.
  - PAM: energy computed transposed (E^T tiles, m on partitions); softmax
    without max-subtraction (energies are small); exp on ScalarE in
    (128,1024) chunks; O = v @ A^T via augmented v^T (ones column gives the
    softmax denominator for free).
  - CAM: per-core partial Gram (64x64) AllGathered; softmax redundant.

Host side: the Bass program and its jitted PJRT executable are built once
per process and cached.  build_program(reps=K) emits the whole computation
K times back-to-back in one NEFF (shared tiles serialize the reps), which
test.py uses to measure the marginal per-execution hardware time.
"""

import os
import sys

sys.path.insert(0, "/opt/trn_rl_repo")

import numpy as np

import concourse.bass as bass
import concourse.mybir as mybir
import concourse.tile as tile
from concourse import bacc
from concourse.masks import make_identity

F32 = mybir.dt.float32
BF16 = mybir.dt.bfloat16
AF = mybir.ActivationFunctionType
ALU = mybir.AluOpType
AX = mybir.AxisListType

NCORES = 8
B = 2
C = 64
HH = 16
N = HH * HH * HH  # 4096
ROW = 18 * 18  # 324, one padded h-slab (w,d padded to 18x18)
LOCPAD = 19  # only w/d deltas (+-18, +-1) can underflow a slab base
LOCVIEW = LOCPAD + 4 * ROW + LOCPAD  # local act view: 4 h-slabs + margins
SLAB = 256  # interior positions per h-slab (16x16)
SHARD = 2 * SLAB  # 512 interior positions per batch per core
SLOPE = (1.0 / 8.0 + 1.0 / 3.0) / 2.0  # RReLU eval negative slope
EPS = 1e-5
NTOT = B * N  # BN normalization count = 8192

AG2_S1 = 2 * B * C * SLAB  # 65536: s1 region elems per rank
AG2_GRAM = B * C * C  # 8192: gram region elems per rank
AG2_PER = AG2_S1 + AG2_GRAM  # 73728

NSLABS_G = NCORES * 2 * B  # halo AG buffer: (rank, slab, batch) blocks


def _deltas():
    out = []
    for dh in (-1, 0, 1):
        for dw in (-1, 0, 1):
            for dd in (-1, 0, 1):
                out.append(dh * ROW + dw * 18 + dd)
    return out


DELTAS = _deltas()
# offset issue order per output slab: halo-dependent offsets last so the
# conv chain starts before the halo AllGather lands.
ORD_LO = list(range(9, 27)) + list(range(0, 9))  # slab 1: dh=-1 last
ORD_HI = list(range(0, 27))  # slab 2: dh=+1 already last
ORD_NAT = (list(range(27)), list(range(27)))


def build_program(reps=1):
    nc = bacc.Bacc(
        "TRN2",
        target_bir_lowering=False,
        debug=False,
        num_devices=NCORES,
    )

    # ---- external inputs (per-core in_maps) ----
    x_loc = nc.dram_tensor("x_loc", [128, LOCVIEW], BF16, kind="ExternalInput")
    w_sc = nc.dram_tensor("w_sc", [128, 27, 128], BF16, kind="ExternalInput")
    w_s1 = nc.dram_tensor("w_s1", [128, 27, 64], BF16, kind="ExternalInput")
    w_c1 = nc.dram_tensor("w_c1", [128, 27, 64], BF16, kind="ExternalInput")
    w_f = nc.dram_tensor("w_f", [128, 27, 64], BF16, kind="ExternalInput")
    qw_d = nc.dram_tensor("qw", [64, 64], BF16, kind="ExternalInput")
    kw_d = nc.dram_tensor("kw", [65, 64], BF16, kind="ExternalInput")
    vw_d = nc.dram_tensor("vw", [65, 66], BF16, kind="ExternalInput")
    bnp_d = nc.dram_tensor("bnp", [128, 9], F32, kind="ExternalInput")
    gam_d = nc.dram_tensor("gam", [1, 2], F32, kind="ExternalInput")
    out_d = nc.dram_tensor("out", [B, C, SHARD], F32, kind="ExternalOutput")

    rg = [list(range(NCORES))]

    with tile.TileContext(nc) as tc:
        dram_cm = tc.tile_pool(name="dram", bufs=1, space="DRAM")
        dram = dram_cm.__enter__()
        # collective input bounce buffers (shared across reps; WAW
        # serializes).  Shared-space AllGather outputs are per rep.
        st1_in = dram.tile([128, 2], F32)
        ag2_in = dram.tile([AG2_PER], F32)
        c2_in = dram.tile([2, B, C, SLAB], BF16)
        s2_in = dram.tile([2, B, C, SLAB], BF16)
        st2_in = dram.tile([64, 4], F32)
        fc_in = dram.tile([2, B, 2 * C, SLAB], BF16)
        stf_in = dram.tile([64, 2], F32)
        bcast_dram = dram.tile([B, SHARD], F32)

        singles_cm = tc.tile_pool(name="singles", bufs=1)
        singles = singles_cm.__enter__()

        ident = singles.tile([64, 64], BF16)
        make_identity(nc, ident[:])
        ident_f32 = singles.tile([64, 64], F32)
        make_identity(nc, ident_f32[:])

        # constants to SBUF
        qw_sb = singles.tile([64, 64], BF16)
        kw_sb = singles.tile([65, 64], BF16)
        vw_sb = singles.tile([65, 66], BF16)
        bnp = singles.tile([128, 9], F32)
        gam_p = singles.tile([1, 1], F32)
        gam_c_col = singles.tile([64, 1], F32)
        nc.sync.dma_start(out=qw_sb[:], in_=qw_d[:])
        nc.sync.dma_start(out=kw_sb[:], in_=kw_d[:])
        nc.sync.dma_start(out=vw_sb[:], in_=vw_d[:])
        nc.sync.dma_start(out=bnp[:], in_=bnp_d[:])
        nc.sync.dma_start(out=gam_p[:], in_=gam_d[0:1, 0:1])
        nc.sync.dma_start(
            out=gam_c_col[:],
            in_=bass.AP(tensor=gam_d, offset=1, ap=[[0, 64], [1, 1]]),
        )
        eps_col = singles.tile([128, 1], F32)
        nc.vector.memset(eps_col[:], EPS)

        # big persistent activations
        acts_cm = tc.tile_pool(name="acts", bufs=1)
        acts = acts_cm.__enter__()
        x_sb = acts.tile([128, LOCVIEW], BF16)
        nc.sync.dma_start(out=x_sb[:], in_=x_loc[:])

        # stacked phase-1 activations: rows 0:64 = s1, 64:128 = c1
        sc1_own = [acts.tile([128, SHARD], F32, name=f"sc1own{b}") for b in range(B)]
        s1_bf = [acts.tile([64, SHARD], BF16, name=f"s1bf{b}") for b in range(B)]
        c1_own = [acts.tile([64, SHARD], F32, name=f"c1own{b}") for b in range(B)]
        c1_own_bf = [acts.tile([64, SHARD], BF16, name=f"c1ownbf{b}") for b in range(B)]

        s1_pam = [acts.tile([65, N], BF16, name=f"s1pam{b}") for b in range(B)]
        for b in range(B):
            nc.vector.memset(s1_pam[b][64:65, :], 1.0)

        k_stack = acts.tile([128, N], BF16)
        q_stack = acts.tile([128, SHARD], BF16)
        vt_sb = [acts.tile([128, 32 * 66], BF16, name=f"vt{b}") for b in range(B)]

        # hoisted per-rep reusable activations (same name => same slot)
        c2both = acts.tile([128, SHARD], BF16)
        s2both = acts.tile([128, SHARD], BF16)
        c2_loc = acts.tile([128, LOCVIEW], BF16)
        s2_loc = acts.tile([128, LOCVIEW], BF16)
        fcat_own = acts.tile([128, B * SHARD], BF16)
        fcat_loc = [acts.tile([128, LOCVIEW], BF16, name=f"fl{b}") for b in range(B)]
        out_own = acts.tile([128, SHARD], F32)
        # zero the padded views once; reps only rewrite interiors
        nc.gpsimd.memset(c2_loc[:], 0.0)
        nc.gpsimd.memset(s2_loc[:], 0.0)
        for b in range(B):
            nc.gpsimd.memset(fcat_loc[b][:], 0.0)

        wpool_cm = tc.tile_pool(name="wpool", bufs=2)
        wpool = wpool_cm.__enter__()

        stats_pool_cm = tc.tile_pool(name="stats", bufs=1)
        stats_pool = stats_pool_cm.__enter__()

        tmp_pool_cm = tc.tile_pool(name="tmp", bufs=2)
        tmp_pool = tmp_pool_cm.__enter__()

        # ---------------- helpers ----------------
        def load_w(dram_t, name, ncols):
            w = wpool.tile([128, 27, ncols], BF16, tag=f"wconv{ncols}", name=name)
            nc.sync.dma_start(out=w[:], in_=dram_t[:])
            return w

        def conv_sc(w_sb_t, act, psum_pool, tname):
            """Column-packed conv S|C on x: stacked (128, 512) raw outputs."""
            touts = []
            stats = []
            for b in range(B):
                t = stats_pool.tile([128, SHARD], F32, name=f"{tname}_t{b}")
                for jj, jslab in enumerate((1, 2)):
                    ps = psum_pool.tile(
                        [128, ROW], F32, tag=f"convps{b}", name=f"{tname}ps{b}{jj}"
                    )
                    base = LOCPAD + jslab * ROW
                    for o in range(27):
                        nc.tensor.matmul(
                            ps[:],
                            lhsT=w_sb_t[64 * b : 64 * b + 64, o, :],
                            rhs=act[
                                64 * b : 64 * b + 64,
                                base + DELTAS[o] : base + DELTAS[o] + ROW,
                            ],
                            start=(o == 0),
                            stop=(o == 26),
                            tile_position=(64 * b, 0),
                        )
                    nc.vector.tensor_copy(
                        t[:, jj * SLAB : (jj + 1) * SLAB],
                        ps[:, :].rearrange("p (w d) -> p w d", w=18)[
                            :, 1:17, 1:17
                        ],
                    )
                touts.append(t)
                ssum = stats_pool.tile([128, 1], F32, name=f"{tname}_s{b}")
                ssq = stats_pool.tile([128, 1], F32, name=f"{tname}_q{b}")
                scr2 = tmp_pool.tile(
                    [128, SHARD], F32, tag="scrB", name=f"{tname}scrB{b}"
                )
                nc.vector.reduce_sum(ssum[:], t[:], axis=AX.X)
                nc.scalar.activation(scr2[:], t[:], AF.Square, accum_out=ssq[:])
                stats.append((ssum, ssq))
            return touts, stats

        def conv64(w_sb_t, act, psum_pool, tname, orders):
            """3x3x3 conv over 64-ch padded local view for own 2 slabs."""
            touts = []
            stats = []
            for b in range(B):
                t = stats_pool.tile([64, SHARD], F32, name=f"{tname}_t{b}")
                for jj, (jslab, order) in enumerate(
                    ((1, orders[0]), (2, orders[1]))
                ):
                    ps = psum_pool.tile(
                        [64, ROW], F32, tag=f"convps{b}", name=f"{tname}ps{b}{jj}"
                    )
                    base = LOCPAD + jslab * ROW
                    for oi, o in enumerate(order):
                        nc.tensor.matmul(
                            ps[:],
                            lhsT=w_sb_t[64 * b : 64 * b + 64, o, :],
                            rhs=act[
                                64 * b : 64 * b + 64,
                                base + DELTAS[o] : base + DELTAS[o] + ROW,
                            ],
                            start=(oi == 0),
                            stop=(oi == 26),
                            tile_position=(64 * b, 0),
                        )
                    nc.vector.tensor_copy(
                        t[:, jj * SLAB : (jj + 1) * SLAB],
                        ps[:, :].rearrange("p (w d) -> p w d", w=18)[
                            :, 1:17, 1:17
                        ],
                    )
                touts.append(t)
                ssum = stats_pool.tile([64, 1], F32, name=f"{tname}_s{b}")
                ssq = stats_pool.tile([64, 1], F32, name=f"{tname}_q{b}")
                scr2 = tmp_pool.tile(
                    [128, SHARD], F32, tag="scrB", name=f"{tname}scrB{b}"
                )
                nc.vector.reduce_sum(ssum[:], t[:], axis=AX.X)
                nc.scalar.activation(
                    scr2[0:64, :], t[:], AF.Square, accum_out=ssq[:]
                )
                stats.append((ssum, ssq))
            return touts, stats

        def conv128(w_sb_t, act_pair, psum_pool, tname, orders):
            """3x3x3 conv with 128 input channels (fused concat), per batch."""
            touts = []
            stats = []
            for b in range(B):
                t = stats_pool.tile([64, SHARD], F32, name=f"{tname}_t{b}")
                for jj, (jslab, order) in enumerate(
                    ((1, orders[0]), (2, orders[1]))
                ):
                    ps = psum_pool.tile(
                        [64, ROW], F32, tag=f"convps{b}", name=f"{tname}ps{b}{jj}"
                    )
                    base = LOCPAD + jslab * ROW
                    for oi, o in enumerate(order):
                        nc.tensor.matmul(
                            ps[:],
                            lhsT=w_sb_t[:, o, :],
                            rhs=act_pair[b][
                                :, base + DELTAS[o] : base + DELTAS[o] + ROW
                            ],
                            start=(oi == 0),
                            stop=(oi == 26),
                        )
                    nc.vector.tensor_copy(
                        t[:, jj * SLAB : (jj + 1) * SLAB],
                        ps[:, :].rearrange("p (w d) -> p w d", w=18)[
                            :, 1:17, 1:17
                        ],
                    )
                touts.append(t)
                ssum = stats_pool.tile([64, 1], F32, name=f"{tname}_s{b}")
                ssq = stats_pool.tile([64, 1], F32, name=f"{tname}_q{b}")
                scr2 = tmp_pool.tile(
                    [128, SHARD], F32, tag="scrB", name=f"{tname}scrB{b}"
                )
                nc.vector.reduce_sum(ssum[:], t[:], axis=AX.X)
                nc.scalar.activation(
                    scr2[0:64, :], t[:], AF.Square, accum_out=ssq[:]
                )
                stats.append((ssum, ssq))
            return touts, stats

        def pack_stats(dst_sb, stats_list):
            for ci, st in enumerate(stats_list):
                (s0, q0), (s1_, q1) = st
                nc.vector.tensor_add(dst_sb[:, 2 * ci : 2 * ci + 1], s0[:], s1_[:])
                nc.vector.tensor_add(
                    dst_sb[:, 2 * ci + 1 : 2 * ci + 2], q0[:], q1[:]
                )

        def bn_coeffs(tot_sb, col, g_col, b_col, name, p=64):
            """A=(g*rstd), B=b-mean*A and the rrelu-scaled variants."""
            mean = stats_pool.tile([p, 1], F32, name=f"{name}_mean")
            var = stats_pool.tile([p, 1], F32, name=f"{name}_var")
            a_t = stats_pool.tile([p, 1], F32, name=f"{name}_A")
            b_t = stats_pool.tile([p, 1], F32, name=f"{name}_B")
            as_t = stats_pool.tile([p, 1], F32, name=f"{name}_As")
            bs_t = stats_pool.tile([p, 1], F32, name=f"{name}_Bs")
            scr = stats_pool.tile([p, 1], F32, name=f"{name}_scr")
            nc.vector.tensor_scalar(
                mean[:], tot_sb[:, col : col + 1], 1.0 / NTOT, None, ALU.mult
            )
            nc.vector.tensor_scalar(
                var[:], tot_sb[:, col + 1 : col + 2], 1.0 / NTOT, None, ALU.mult
            )
            nc.vector.tensor_mul(scr[:], mean[:], mean[:])
            nc.vector.tensor_sub(var[:], var[:], scr[:])
            # rstd = exp(-0.5*ln(var+eps)); avoids the Sqrt table set
            nc.scalar.activation(scr[:], var[:], AF.Ln, bias=eps_col[0:p, :])
            nc.vector.tensor_scalar(scr[:], scr[:], -0.5, None, ALU.mult)
            nc.scalar.activation(scr[:], scr[:], AF.Exp)
            nc.vector.tensor_mul(a_t[:], scr[:], g_col)
            nc.vector.tensor_mul(scr[:], mean[:], a_t[:])
            nc.vector.tensor_sub(b_t[:], b_col, scr[:])
            nc.vector.tensor_scalar(as_t[:], a_t[:], SLOPE, None, ALU.mult)
            nc.vector.tensor_scalar(bs_t[:], b_t[:], SLOPE, None, ALU.mult)
            return a_t, b_t, as_t, bs_t

        def bn_rrelu(t_raw, coeffs, dst_ap, p=64):
            """dst = max(A*t+B, As*t+Bs) elementwise."""
            a_t, b_t, as_t, bs_t = coeffs
            y1 = tmp_pool.tile([128, SHARD], F32, tag="y1", name="y1_t")
            y2 = tmp_pool.tile([128, SHARD], F32, tag="y2", name="y2_t")
            nc.vector.tensor_scalar(
                y1[0:p, :], t_raw[:], a_t[:], b_t[:], ALU.mult, ALU.add
            )
            nc.vector.tensor_scalar(
                y2[0:p, :], t_raw[:], as_t[:], bs_t[:], ALU.mult, ALU.add
            )
            nc.vector.tensor_max(dst_ap, y1[0:p, :], y2[0:p, :])

        def own_fill(dst, own_ap):
            nc.vector.tensor_copy(
                dst[:, LOCPAD + 1 * ROW : LOCPAD + 3 * ROW]
                .rearrange("p (j w d) -> p j w d", j=2, w=18)[:, :, 1:17, 1:17],
                own_ap,
            )

        def halo_fill(ag_t, dsts, rv, name):
            """dst halo interiors <- neighbours' slabs via dynamic DMAs.

            ag_t: DRAM tile logically [NSLABS_G, nrows, SLAB]; block
            (rank r, slab s, batch b) at index (2r+s)*B+b.
            dsts: per-batch (dst_view, row0, nrows)."""
            for dslab, cond in ((0, rv >= 1), (3, rv <= NCORES - 2)):
                for b, (dst, row0, nrows) in enumerate(dsts):
                    interior = (
                        dst[
                            row0 : row0 + nrows,
                            LOCPAD + dslab * ROW : LOCPAD + (dslab + 1) * ROW,
                        ]
                        .rearrange("p (w d) -> p w d", w=18)[:, 1:17, 1:17]
                    )
                    nc.vector.memset(interior, 0.0)
                    if dslab == 0:
                        # lo halo: rank rv-1, slab 1
                        g = ((rv * 2 + 2 * NCORES - 1) * B + b) % NSLABS_G
                    else:
                        # hi halo: rank rv+1, slab 0
                        g = (((rv + 1) * 2) * B + b) % NSLABS_G
                    nc.sync.dma_start(
                        out=interior, in_=ag_t[:].tensor[g], cond=cond
                    )

        def emit_body(rep):
            # per-rep Shared collective outputs (single-writer rule)
            st1_out = dram.tile([NCORES, 128, 2], F32, addr_space="Shared",
                                name=f"st1_out_r{rep}")
            ag2_out = dram.tile([NCORES * AG2_PER], F32, addr_space="Shared",
                                name=f"ag2_out_r{rep}")
            c2_ag = dram.tile([NSLABS_G, C, SLAB], BF16,
                              addr_space="Shared", name=f"c2_ag_r{rep}")
            s2_ag = dram.tile([NSLABS_G, C, SLAB], BF16,
                              addr_space="Shared", name=f"s2_ag_r{rep}")
            st2_out = dram.tile([NCORES, 64, 4], F32, addr_space="Shared",
                                name=f"st2_out_r{rep}")
            fc_ag = dram.tile([NSLABS_G, 2 * C, SLAB], BF16,
                              addr_space="Shared", name=f"fc_ag_r{rep}")
            stf_out = dram.tile([NCORES, 64, 2], F32, addr_space="Shared",
                                name=f"stf_out_r{rep}")

            rv = nc.sync.partition_id()

            # =========== phase 1: conv S|C (column-packed) ===========
            cpsum_cm = tc.tile_pool(name=f"cpsum_r{rep}", bufs=2, space="PSUM")
            cpsum = cpsum_cm.__enter__()

            wsc_sb = load_w(w_sc, "wsSC", 128)
            tSC, statSC = conv_sc(wsc_sb, x_sb, cpsum, "cSC")

            st1_sb = stats_pool.tile([128, 2], F32, name="st1_sb")
            pack_stats(st1_sb, [statSC])
            nc.sync.dma_start(out=st1_in[:], in_=st1_sb[:])
            nc.gpsimd.collective_compute(
                "AllGather",
                ALU.bypass,
                replica_groups=rg,
                ins=[st1_in[:].opt()],
                outs=[st1_out[:].opt()],
            )

            st1_stage = stats_pool.tile([128, 2, NCORES], F32, name="st1_stage")
            nc.sync.dma_start(
                out=st1_stage[:],
                in_=bass.AP(
                    tensor=st1_out[:].tensor,
                    offset=0,
                    ap=[[2, 128], [1, 2], [256, NCORES]],
                ),
            )
            st1_tot = stats_pool.tile([128, 2], F32, name="st1_tot")
            nc.vector.tensor_reduce(st1_tot[:], st1_stage[:], axis=AX.X, op=ALU.add)
            cSC = bn_coeffs(st1_tot, 0, bnp[:, 0:1], bnp[:, 1:2], "bnSC", p=128)

            for b in range(B):
                bn_rrelu(tSC[b], cSC, sc1_own[b][:], p=128)
                nc.vector.tensor_copy(s1_bf[b][:], sc1_own[b][0:64, :])
                nc.vector.tensor_copy(c1_own[b][:], sc1_own[b][64:128, :])
                nc.vector.tensor_copy(c1_own_bf[b][:], sc1_own[b][64:128, :])

            cpsum_cm.__exit__(None, None, None)

            # =========== phase 2: CAM partial gram + AG2 (s1 + gram) ===========
            mpsum_cm = tc.tile_pool(name=f"mpsum_r{rep}", bufs=2, space="PSUM")
            mpsum = mpsum_cm.__enter__()

            ft_sb = [tmp_pool.tile([128, 4 * 64], BF16, tag=f"ft{b}", name=f"ft{b}") for b in range(B)]
            gram_sb = tmp_pool.tile([64, B * 64], F32, tag="gram", name="gram_sb")
            for b in range(B):
                for kk in range(4):
                    pst = mpsum.tile([128, 64], BF16, tag="mm", name=f"ft{b}{kk}")
                    nc.tensor.transpose(
                        pst[:],
                        c1_own_bf[b][:, 128 * kk : 128 * (kk + 1)],
                        ident[:],
                    )
                    nc.vector.tensor_copy(
                        ft_sb[b][:, 64 * kk : 64 * (kk + 1)], pst[:, 0:64]
                    )
                psg = mpsum.tile([64, 64], F32, tag="mm", name=f"gram{b}")
                for kk in range(4):
                    nc.tensor.matmul(
                        psg[:],
                        lhsT=ft_sb[b][:, 64 * kk : 64 * (kk + 1)],
                        rhs=ft_sb[b][:, 64 * kk : 64 * (kk + 1)],
                        start=(kk == 0),
                        stop=(kk == 3),
                    )
                nc.vector.tensor_copy(gram_sb[:, 64 * b : 64 * (b + 1)], psg[:])

            # write AG2 contribution: s1 (slab-major) + gram
            for b in range(B):
                nc.sync.dma_start(
                    out=bass.AP(
                        tensor=ag2_in[:].tensor,
                        offset=b * C * SLAB,
                        ap=[[SLAB, 64], [B * C * SLAB, 2], [1, SLAB]],
                    ),
                    in_=sc1_own[b][0:64, :].rearrange("p (j s) -> p j s", j=2),
                )
            nc.sync.dma_start(
                out=bass.AP(
                    tensor=ag2_in[:].tensor,
                    offset=AG2_S1,
                    ap=[[64, 64], [64 * 64, B], [1, 64]],
                ),
                in_=gram_sb[:].rearrange("p (b c) -> p b c", b=B),
            )
            nc.gpsimd.collective_compute(
                "AllGather",
                ALU.bypass,
                replica_groups=rg,
                ins=[ag2_in[:].opt()],
                outs=[ag2_out[:].opt()],
            )

            # =========== phase 3: q (local), then k/vT from gathered s1 ===========
            for b in range(B):
                psq = mpsum.tile([64, SHARD], F32, tag="qk", name=f"q{b}")
                nc.tensor.matmul(
                    psq[:],
                    lhsT=qw_sb[:],
                    rhs=s1_bf[b][:],
                    start=True,
                    stop=True,
                )
                # + qb (bnp rows 64:128 col 4)
                nc.vector.tensor_scalar(
                    q_stack[64 * b : 64 * (b + 1), :],
                    psq[:],
                    bnp[0:64, 8:9],
                    None,
                    ALU.add,
                )

            # load gathered s1 into s1_pam (global n order)
            for b in range(B):
                for j in range(2):
                    nc.gpsimd.dma_start(
                        out=s1_pam[b][0:64, :]
                        .rearrange("p (g s) -> p g s", s=2 * SLAB)[:, :, j * SLAB : (j + 1) * SLAB],
                        in_=bass.AP(
                            tensor=ag2_out[:].tensor,
                            offset=b * C * SLAB + j * B * C * SLAB,
                            ap=[[SLAB, 64], [AG2_PER, NCORES], [1, SLAB]],
                        ),
                    )
            # gathered gram -> reduce over cores
            gram_full = [tmp_pool.tile([64, 64], F32, tag=f"gramf{b}", name=f"gramf{b}") for b in range(B)]
            for b in range(B):
                gstage = tmp_pool.tile(
                    [64, 64, NCORES], F32, tag="gstage", name=f"gstage{b}"
                )
                nc.sync.dma_start(
                    out=gstage[:],
                    in_=bass.AP(
                        tensor=ag2_out[:].tensor,
                        offset=AG2_S1 + b * C * C,
                        ap=[[64, 64], [1, 64], [AG2_PER, NCORES]],
                    ),
                )
                nc.vector.tensor_reduce(gram_full[b][:], gstage[:], axis=AX.X, op=ALU.add)

            for b in range(B):
                for nt in range(8):
                    psk = mpsum.tile([64, 512], F32, tag="qk", name=f"k{b}{nt}")
                    nc.tensor.matmul(
                        psk[:],
                        lhsT=kw_sb[:],
                        rhs=s1_pam[b][:, 512 * nt : 512 * (nt + 1)],
                        start=True,
                        stop=True,
                    )
                    nc.vector.tensor_copy(
                        k_stack[64 * b : 64 * (b + 1), 512 * nt : 512 * (nt + 1)],
                        psk[:],
                    )
                for mt in range(32):
                    psv = mpsum.tile([128, 66], F32, tag="vt", name=f"v{b}{mt}")
                    nc.tensor.matmul(
                        psv[:],
                        lhsT=s1_pam[b][:, 128 * mt : 128 * (mt + 1)],
                        rhs=vw_sb[:],
                        start=True,
                        stop=True,
                    )
                    nc.vector.tensor_copy(
                        vt_sb[b][:, 66 * mt : 66 * (mt + 1)], psv[:]
                    )

            # =========== phase 4: CAM finish -> c2 -> halo AG ===========
            for b in range(B):
                rowmax = tmp_pool.tile([64, 1], F32, tag="camx", name=f"camx{b}")
                den = tmp_pool.tile([64, 1], F32, tag="camd", name=f"camd{b}")
                attn = tmp_pool.tile([64, 64], F32, tag="cama", name=f"cama{b}")
                nc.vector.tensor_reduce(
                    rowmax[:], gram_full[b][:], axis=AX.X, op=ALU.min
                )
                nc.scalar.activation(
                    attn[:],
                    gram_full[b][:],
                    AF.Exp,
                    bias=rowmax[:],
                    scale=-1.0,
                    accum_out=den[:],
                )
                nc.vector.reciprocal(den[:], den[:])
                nc.vector.tensor_scalar(attn[:], attn[:], den[:], None, ALU.mult)
                # attn^T via PE
                psat = mpsum.tile([64, 64], F32, tag="mm", name=f"at{b}")
                nc.tensor.transpose(psat[:], attn[:], ident_f32[:])
                attnT = tmp_pool.tile([64, 64], BF16, tag="camat", name=f"camat{b}")
                nc.vector.tensor_copy(attnT[:], psat[:])
                # cam_out = attnT.T @ c1_own
                psco = mpsum.tile([64, SHARD], F32, tag="qk", name=f"co{b}")
                nc.tensor.matmul(
                    psco[:],
                    lhsT=attnT[:],
                    rhs=c1_own_bf[b][:],
                    start=True,
                    stop=True,
                )
                c2t = tmp_pool.tile([64, SHARD], F32, tag="c2t", name=f"c2t{b}")
                nc.vector.tensor_scalar(c2t[:], psco[:], gam_c_col[:, 0:1], None, ALU.mult)
                nc.vector.tensor_add(
                    c2both[64 * b : 64 * (b + 1), :], c2t[:], c1_own[b][:]
                )
                nc.sync.dma_start(
                    out=bass.AP(
                        tensor=c2_in[:].tensor,
                        offset=b * C * SLAB,
                        ap=[[SLAB, 64], [B * C * SLAB, 2], [1, SLAB]],
                    ),
                    in_=c2both[64 * b : 64 * (b + 1), :].rearrange(
                        "p (j s) -> p j s", j=2
                    ),
                )
            nc.gpsimd.collective_compute(
                "AllGather", ALU.bypass, replica_groups=rg,
                ins=[c2_in[:].opt()], outs=[c2_ag[:].opt()],
            )

            mpsum_cm.__exit__(None, None, None)

            # =========== phase 5: PAM attention ===========
            epsum_cm = tc.tile_pool(name=f"epsum_r{rep}", bufs=3, space="PSUM")
            epsum = epsum_cm.__enter__()
            opsum_cm = tc.tile_pool(name=f"opsum_r{rep}", bufs=1, space="PSUM")
            opsum = opsum_cm.__enter__()
            apool_cm = tc.tile_pool(name=f"apool_r{rep}", bufs=3)
            apool = apool_cm.__enter__()

            o_ps = [
                opsum.tile([65, SHARD], F32, name=f"ops{b}", tag=f"ops{b}")
                for b in range(B)
            ]
            for g2 in range(16):
                for b in range(B):
                    e_ps = epsum.tile([128, 1024], F32, tag="eg", name=f"e{g2}{b}")
                    for j in range(2):
                        mt = 2 * g2 + j
                        nc.tensor.matmul(
                            e_ps[:, 512 * j : 512 * (j + 1)],
                            lhsT=k_stack[
                                64 * b : 64 * (b + 1), 128 * mt : 128 * (mt + 1)
                            ],
                            rhs=q_stack[64 * b : 64 * (b + 1), :],
                            start=True,
                            stop=True,
                            tile_position=(64 * b, 0),
                        )
                    a_sb = apool.tile([128, 1024], BF16, tag="ag", name=f"a{g2}{b}")
                    nc.scalar.activation(a_sb[:], e_ps[:], AF.Exp)
                    for j in range(2):
                        mt = 2 * g2 + j
                        nc.tensor.matmul(
                            o_ps[b][:],
                            lhsT=vt_sb[b][:, 66 * mt : 66 * mt + 65],
                            rhs=a_sb[:, 512 * j : 512 * (j + 1)],
                            start=(mt == 0),
                            stop=(mt == 31),
                        )

            # =========== phase 6: PAM finalize -> s2 -> halo AG ===========
            for b in range(B):
                recip = tmp_pool.tile([1, SHARD], F32, tag="rec", name=f"rec{b}")
                recipg = tmp_pool.tile([1, SHARD], F32, tag="recg", name=f"recg{b}")
                nc.vector.reciprocal(recip[:], o_ps[b][64:65, :])
                nc.vector.tensor_scalar(
                    recipg[:], recip[:], gam_p[:, 0:1], None, ALU.mult
                )
                nc.sync.dma_start(out=bcast_dram[b : b + 1, :], in_=recipg[:])
                bc_sb = tmp_pool.tile([64, SHARD], F32, tag="bcs", name=f"bcs{b}")
                nc.sync.dma_start(
                    out=bc_sb[:],
                    in_=bass.AP(
                        tensor=bcast_dram[:].tensor,
                        offset=b * SHARD,
                        ap=[[0, 64], [1, SHARD]],
                    ),
                )
                s2t = tmp_pool.tile([64, SHARD], F32, tag="s2t", name=f"s2t{b}")
                nc.vector.tensor_mul(s2t[:], o_ps[b][0:64, :], bc_sb[:])
                nc.vector.tensor_add(
                    s2both[64 * b : 64 * (b + 1), :], s2t[:], sc1_own[b][0:64, :]
                )
                nc.sync.dma_start(
                    out=bass.AP(
                        tensor=s2_in[:].tensor,
                        offset=b * C * SLAB,
                        ap=[[SLAB, 64], [B * C * SLAB, 2], [1, SLAB]],
                    ),
                    in_=s2both[64 * b : 64 * (b + 1), :].rearrange(
                        "p (j s) -> p j s", j=2
                    ),
                )
            nc.gpsimd.collective_compute(
                "AllGather", ALU.bypass, replica_groups=rg,
                ins=[s2_in[:].opt()], outs=[s2_ag[:].opt()],
            )

            for p in (apool_cm, opsum_cm, epsum_cm):
                p.__exit__(None, None, None)
            cpsum_cm = tc.tile_pool(name=f"cpsum2_r{rep}", bufs=2, space="PSUM")
            cpsum = cpsum_cm.__enter__()

            # =========== phase 7: conv C1 (on gathered c2) ===========
            own_fill(
                c2_loc,
                c2both[:].rearrange("p (j w d) -> p j w d", j=2, w=16),
            )
            halo_fill(c2_ag, [(c2_loc, 0, 64), (c2_loc, 64, 64)], rv, "c2")
            wc1_sb = load_w(w_c1, "wsC1", 64)
            tC1, statC1 = conv64(wc1_sb, c2_loc, cpsum, "cC1", (ORD_LO, ORD_HI))

            # =========== phase 8: conv S1 (on gathered s2) ===========
            own_fill(
                s2_loc,
                s2both[:].rearrange("p (j w d) -> p j w d", j=2, w=16),
            )
            halo_fill(s2_ag, [(s2_loc, 0, 64), (s2_loc, 64, 64)], rv, "s2")
            ws1_sb = load_w(w_s1, "wsS1", 64)
            tS1, statS1 = conv64(ws1_sb, s2_loc, cpsum, "cS1", (ORD_LO, ORD_HI))

            st2_sb = stats_pool.tile([64, 4], F32, name="st2_sb")
            pack_stats(st2_sb, [statS1, statC1])
            nc.sync.dma_start(out=st2_in[:], in_=st2_sb[:])
            nc.gpsimd.collective_compute(
                "AllGather",
                ALU.bypass,
                replica_groups=rg,
                ins=[st2_in[:].opt()],
                outs=[st2_out[:].opt()],
            )
            st2_stage = stats_pool.tile([64, 4, NCORES], F32, name="st2_stage")
            nc.sync.dma_start(
                out=st2_stage[:],
                in_=bass.AP(
                    tensor=st2_out[:].tensor,
                    offset=0,
                    ap=[[4, 64], [1, 4], [256, NCORES]],
                ),
            )
            st2_tot = stats_pool.tile([64, 4], F32, name="st2_tot")
            nc.vector.tensor_reduce(st2_tot[:], st2_stage[:], axis=AX.X, op=ALU.add)
            cS1 = bn_coeffs(st2_tot, 0, bnp[0:64, 2:3], bnp[0:64, 3:4], "bnS1")
            cC1 = bn_coeffs(st2_tot, 2, bnp[0:64, 4:5], bnp[0:64, 5:6], "bnC1")

            for b in range(B):
                bn_rrelu(tS1[b], cS1, fcat_own[0:64, b * SHARD : (b + 1) * SHARD])
                bn_rrelu(tC1[b], cC1, fcat_own[64:128, b * SHARD : (b + 1) * SHARD])

            # contribution: (2 slab, 2 b, 128 c, 256); one DMA per batch
            for b in range(B):
                nc.sync.dma_start(
                    out=bass.AP(
                        tensor=fc_in[:].tensor,
                        offset=b * 2 * C * SLAB,
                        ap=[[SLAB, 128], [B * 2 * C * SLAB, 2], [1, SLAB]],
                    ),
                    in_=fcat_own[:, b * SHARD : (b + 1) * SHARD].rearrange(
                        "p (j s) -> p j s", j=2
                    ),
                )
            nc.gpsimd.collective_compute(
                "AllGather", ALU.bypass, replica_groups=rg,
                ins=[fc_in[:].opt()], outs=[fc_ag[:].opt()],
            )

            # =========== phase 9: conv F ===========
            for b in range(B):
                own_fill(
                    fcat_loc[b],
                    fcat_own[:, b * SHARD : (b + 1) * SHARD].rearrange(
                        "p (j w d) -> p j w d", j=2, w=16
                    ),
                )
            halo_fill(
                fc_ag, [(fcat_loc[0], 0, 128), (fcat_loc[1], 0, 128)], rv, "fc"
            )
            wf_sb = load_w(w_f, "wsF", 64)
            tF, statF = conv128(wf_sb, fcat_loc, cpsum, "cF", (ORD_LO, ORD_HI))

            stf_sb = stats_pool.tile([64, 2], F32, name="stf_sb")
            pack_stats(stf_sb, [statF])
            nc.sync.dma_start(out=stf_in[:], in_=stf_sb[:])
            nc.gpsimd.collective_compute(
                "AllGather",
                ALU.bypass,
                replica_groups=rg,
                ins=[stf_in[:].opt()],
                outs=[stf_out[:].opt()],
            )
            stf_stage = stats_pool.tile([64, 2, NCORES], F32, name="stf_stage")
            nc.sync.dma_start(
                out=stf_stage[:],
                in_=bass.AP(
                    tensor=stf_out[:].tensor,
                    offset=0,
                    ap=[[2, 64], [1, 2], [128, NCORES]],
                ),
            )
            stf_tot = stats_pool.tile([64, 2], F32, name="stf_tot")
            nc.vector.tensor_reduce(stf_tot[:], stf_stage[:], axis=AX.X, op=ALU.add)
            cF = bn_coeffs(stf_tot, 0, bnp[0:64, 6:7], bnp[0:64, 7:8], "bnF")

            for b in range(B):
                bn_rrelu(tF[b], cF, out_own[64 * b : 64 * (b + 1), :])
            nc.sync.dma_start(
                out=bass.AP(
                    tensor=out_d,
                    offset=0,
                    ap=[[SHARD, 128], [1, SHARD]],
                ),
                in_=out_own[:],
            )
            cpsum_cm.__exit__(None, None, None)

        for rep in range(reps):
            emit_body(rep)

        for p in (tmp_pool_cm, stats_pool_cm, wpool_cm, acts_cm,
                  singles_cm, dram_cm):
            p.__exit__(None, None, None)

    nc.finalize()
    return nc


def _prep_host(inputs):
    """Build per-core in_maps from the full problem inputs."""
    x = np.asarray(inputs["x"], np.float32)

    import ml_dtypes

    def conv_wT(w):
        # w: (O, I, 3, 3, 3) -> (I, 27, O)
        return np.transpose(np.asarray(w, np.float32), (1, 2, 3, 4, 0)).reshape(
            w.shape[1], 27, w.shape[0]
        )

    def dup_rows(wt):
        return np.concatenate([wt, wt], axis=0)

    wsT = conv_wT(inputs["wS"])
    wcT = conv_wT(inputs["wC"])
    w_sc = dup_rows(np.concatenate([wsT, wcT], axis=2))  # (128, 27, 128)

    qw = np.asarray(inputs["qw"], np.float32).reshape(64, 64)
    kw = np.asarray(inputs["kw"], np.float32).reshape(64, 64)
    vw = np.asarray(inputs["vw"], np.float32).reshape(64, 64)
    ka = np.zeros((65, 64), np.float32)
    ka[:64] = kw.T
    ka[64] = np.asarray(inputs["kb"], np.float32)
    va = np.zeros((65, 66), np.float32)
    va[:64, :64] = vw.T
    va[64, :64] = np.asarray(inputs["vb"], np.float32)
    va[64, 64] = 1.0

    bnp = np.zeros((128, 9), np.float32)
    bnp[0:64, 0] = np.asarray(inputs["gS"], np.float32)
    bnp[64:128, 0] = np.asarray(inputs["gC"], np.float32)
    bnp[0:64, 1] = np.asarray(inputs["bS"], np.float32)
    bnp[64:128, 1] = np.asarray(inputs["bC"], np.float32)
    for col, k in enumerate(("gS1", "bS1", "gC1", "bC1", "gF", "bF", "qb")):
        bnp[0:64, col + 2] = np.asarray(inputs[k], np.float32)

    gam = np.array(
        [[float(np.asarray(inputs["gamma_p"]).reshape(-1)[0]),
          float(np.asarray(inputs["gamma_c"]).reshape(-1)[0])]],
        np.float32,
    )

    shared = {
        "w_sc": w_sc.astype(ml_dtypes.bfloat16),
        "w_s1": dup_rows(conv_wT(inputs["wS1"])).astype(ml_dtypes.bfloat16),
        "w_c1": dup_rows(conv_wT(inputs["wC1"])).astype(ml_dtypes.bfloat16),
        "w_f": conv_wT(inputs["wF"]).astype(ml_dtypes.bfloat16),
        "qw": qw.T.astype(ml_dtypes.bfloat16),
        "kw": ka.astype(ml_dtypes.bfloat16),
        "vw": va.astype(ml_dtypes.bfloat16),
        "bnp": np.ascontiguousarray(bnp),
        "gam": gam,
    }

    # padded full volume (h, w, d) -> (18, 18, 18), flattened per (b, c)
    xp = np.zeros((B, C, 18, 18, 18), np.float32)
    xp[:, :, 1:17, 1:17, 1:17] = x
    xp = xp.reshape(B, C, 18, ROW)

    in_maps = []
    for i in range(NCORES):
        xl = np.zeros((128, LOCVIEW), np.float32)
        for j in range(4):
            g = 2 * i - 1 + j  # global h-slab (padded index g+1)
            view = xp[:, :, g + 1].reshape(128, ROW)
            xl[:, LOCPAD + j * ROW : LOCPAD + (j + 1) * ROW] = view
        m = dict(shared)
        m["x_loc"] = xl.astype(ml_dtypes.bfloat16)
        in_maps.append(m)
    return in_maps


class _Exec:
    """Compile-once executor: bass program + cached jitted PJRT callable."""

    def __init__(self, reps):
        import jax
        from jax.sharding import Mesh, PartitionSpec, NamedSharding
        from jax.experimental.shard_map import shard_map
        from concourse.bass2jax import (
            _bass_exec_p,
            install_neuronx_cc_hook,
            partition_id_tensor,
        )

        self.jax = jax
        self.nc = build_program(reps)
        install_neuronx_cc_hook()
        nc_ = self.nc
        partition_name = (
            nc_.partition_id_tensor.name if nc_.partition_id_tensor else None
        )
        in_names, out_names, out_avals, zero_shapes = [], [], [], []
        for alloc in nc_.m.functions[0].allocations:
            if not isinstance(alloc, mybir.MemoryLocationSet):
                continue
            name = alloc.memorylocations[0].name
            if alloc.kind == "ExternalInput":
                if name != partition_name:
                    in_names.append(name)
            elif alloc.kind == "ExternalOutput":
                shape = tuple(alloc.tensor_shape)
                dtype = mybir.dt.np(alloc.dtype)
                out_names.append(name)
                out_avals.append(jax.core.ShapedArray(shape, dtype))
                zero_shapes.append((shape, dtype))
        self.in_names = in_names
        self.out_names = out_names
        self.out_avals = out_avals
        self.zero_shapes = zero_shapes
        n_params = len(in_names)
        n_outs = len(out_avals)
        self.n_params = n_params
        all_in = in_names + out_names + ([partition_name] if partition_name else [])

        def _body(*args):
            operands = list(args)
            if partition_name:
                operands.append(partition_id_tensor())
            return tuple(
                _bass_exec_p.bind(
                    *operands,
                    out_avals=tuple(out_avals),
                    in_names=tuple(all_in),
                    out_names=tuple(out_names),
                    lowering_input_output_aliases=(),
                    sim_require_finite=True,
                    sim_require_nnan=True,
                    nc=nc_,
                )
            )

        devices = jax.devices()[:NCORES]
        assert len(devices) == NCORES
        self.mesh = Mesh(np.asarray(devices), ("core",))
        self.psharded = NamedSharding(self.mesh, PartitionSpec("core"))
        self.sharded = jax.jit(
            shard_map(
                _body,
                mesh=self.mesh,
                in_specs=(PartitionSpec("core"),) * (n_params + n_outs),
                out_specs=(PartitionSpec("core"),) * n_outs,
                check_rep=False,
            ),
            donate_argnums=tuple(range(n_params, n_params + n_outs)),
            keep_unused=True,
        )

    def concat_inputs(self, in_maps):
        return [
            np.concatenate([np.asarray(m[name]) for m in in_maps], axis=0)
            for name in self.in_names
        ]

    def zeros(self):
        return [
            np.zeros((NCORES * s[0], *s[1:]), d) for (s, d) in self.zero_shapes
        ]

    def run(self, in_maps):
        """Full path: numpy in -> per-core dict of numpy outputs."""
        concat_in = self.concat_inputs(in_maps)
        out_arrs = self.sharded(*concat_in, *self.zeros())
        return [
            {
                name: np.asarray(out_arrs[i]).reshape(
                    NCORES, *self.out_avals[i].shape
                )[c]
                for i, name in enumerate(self.out_names)
            }
            for c in range(NCORES)
        ]


_EXECS = {}


def _get_exec(reps=1):
    if reps not in _EXECS:
        _EXECS[reps] = _Exec(reps)
    return _EXECS[reps]


def _run_fallback(inputs):
    """Stock path for non-axon environments."""
    from concourse.bass_utils import run_bass_kernel_spmd

    if "nc" not in _EXECS:
        _EXECS["nc"] = build_program(1)
    nc_ = _EXECS["nc"]
    in_maps = _prep_host(inputs)
    return run_bass_kernel_spmd(nc_, in_maps, list(range(NCORES))).results


def kernel(**inputs) -> np.ndarray:
    from concourse._compat import axon_active

    if axon_active():
        ex = _get_exec(1)
        res = ex.run(_prep_host(inputs))
    else:
        res = _run_fallback(inputs)
    out = np.zeros((B, C, HH, HH, HH), np.float32)
    ov = out.reshape(B, C, 8, 2, SLAB)
    for i in range(NCORES):
        ov[:, :, i] = res[i]["out"].reshape(B, C, 2, SLAB)
    return out


if __name__ == "__main__":
    rng = np.random.default_rng(0)
    print("building program...")
    nc = build_program()
    print("ok")
